# revision 1
# baseline (speedup 1.0000x reference)
"""GCMC message-passing kernel for trn2: builder + host preprocessing.

Per core = one dst-shard, both directions (0: drug->dis, 1: dis->drug).
  Phase W: device computes W[r] = att @ basis -> Wtab[R, IN*MU] f32 in HBM.
  Phase E (x6 passes = 2 dirs x 3 k-feats): per-edge event streams sorted by
    slot (r-major, dst-local), 128-event windows, WPP windows per 128-slot
    page. dma_gather pulls 64-f32 W rows (1024 events/call); DVE builds
    SegT[128ev,128slot] = is_equal(IC, sl) * sc  (sc = cj*ci, host-folded);
    PE: msgs.T @ SegT accumulated into a PSUM page [MU, 128].
    Pages -> SBUF stage (ACT) -> hT[d,k] = [MU, NSLOT] f32 HBM (SYNC).
  Phase P: outT[d] [256, SH] = sum_rk fcblk_rk.T @ hT-slices + bias.
Host assembles + transposes the two outputs.
"""
import numpy as np
import concourse.bass as bass
import concourse.bacc as bacc
import concourse.mybir as mybir

F32 = mybir.dt.float32
I16 = mybir.dt.int16

R = 5
MU = 64
OUT = 256
NK = 3


class Cfg:
    def __init__(self, n_nodes, in_units, n_cores, wpp):
        self.N = n_nodes
        self.IN = in_units
        self.NC = n_cores
        self.SH = n_nodes // n_cores
        self.PPR = (self.SH + 127) // 128
        self.NPAGE = R * self.PPR
        self.NSLOT = self.NPAGE * 128
        self.WPP = wpp
        self.NW = self.NPAGE * wpp
        self.NWP = ((self.NW + 7) // 8) * 8
        self.NCALL = self.NWP // 8
        self.CPC = 8                          # gather calls per input chunk
        self.WPC = self.CPC * 8               # windows per chunk
        self.NCHUNK = (self.NCALL + self.CPC - 1) // self.CPC
        self.DCH = 512
        self.NDC = (self.SH + self.DCH - 1) // self.DCH
        self.SPS = 16
        self.NSTG = (self.NPAGE + self.SPS - 1) // self.SPS
        self.WROUND = (in_units * MU) // 2048
        assert (in_units * MU) % 2048 == 0



def host_prep_streams(cfg, feat_src, cj_src, ci_dst, src, dst, core):
    SH, PPR, WPP = cfg.SH, cfg.PPR, cfg.WPP
    lo, hi = core * SH, (core + 1) * SH
    ev = [[[], [], []] for _ in range(NK)]
    for r in range(R):
        m = (dst[r] >= lo) & (dst[r] < hi)
        s, d = src[r][m], dst[r][m]
        slot = r * PPR * 128 + (d - lo)
        sc = (cj_src[s, 0] * ci_dst[d, 0]).astype(np.float32)
        for k in range(NK):
            ev[k][0].append((r * cfg.IN + feat_src[s, k]).astype(np.int64))
            ev[k][1].append(slot.astype(np.int64))
            ev[k][2].append(sc)
    out = {}
    for k in range(NK):
        g = np.concatenate(ev[k][0])
        sl = np.concatenate(ev[k][1])
        sc = np.concatenate(ev[k][2])
        order = np.argsort(sl, kind="stable")
        g, sl, sc = g[order], sl[order], sc[order]
        NWP = cfg.NWP
        G = np.zeros((NWP, 128), np.int16)
        SL = np.zeros((NWP, 128), np.float32)
        SC = np.zeros((NWP, 128), np.float32)
        page = sl // 128
        counts = np.bincount(page, minlength=cfg.NPAGE)
        assert counts.max() <= WPP * 128, (
            f"page overflow {counts.max()} > {WPP*128}; raise WPP")
        pos = 0
        for p in range(cfg.NPAGE):
            n = counts[p]
            gs, sls, scs = g[pos:pos+n], sl[pos:pos+n], sc[pos:pos+n]
            pos += n
            for w in range((n + 127) // 128):
                a, b = w * 128, min((w + 1) * 128, n)
                wi = p * WPP + w
                G[wi, :b-a] = gs[a:b]
                SL[wi, :b-a] = (sls[a:b] - p * 128).astype(np.float32)
                SC[wi, :b-a] = scs[a:b]
        Gc = G.reshape(cfg.NCALL, 1024)
        wr = np.zeros((16, cfg.NCALL * 64), np.int16)
        for c in range(cfg.NCALL):
            wr[:, c*64:(c+1)*64] = Gc[c].reshape(64, 16).T
        out[f"g{k}"] = np.tile(wr, (8, 1))
        out[f"sl{k}"] = SL.T.copy()
        out[f"sc{k}"] = SC.T.copy()
    return out


def build_inputs(cfg, inputs):
    f32 = np.float32
    gi = lambda n: np.asarray(inputs[n], np.int64)
    gf = lambda n: np.asarray(inputs[n], f32)
    drug_feat, dis_feat = gi("drug_feat"), gi("dis_feat")
    src, dst = gi("src"), gi("dst")
    cj_drug, ci_drug = gf("cj_drug"), gf("ci_drug")
    cj_dis, ci_dis = gf("cj_dis"), gf("ci_dis")
    att, basis = gf("att"), gf("basis")
    fc_w, fc_b = gf("fc_w"), gf("fc_b")

    attT = att.T.copy()
    basisf = basis.reshape(4, cfg.IN * MU).copy()
    # fcrT[m, rk, o] = fc_w[r*NK*MU + k*MU + m, o]
    fcrT = fc_w.reshape(R * NK, MU, OUT).transpose(1, 0, 2).copy()
    fcb2 = fc_b.reshape(2, 128).T.copy()      # [128, 2], col h = half h
    IC = np.tile(np.arange(128, dtype=f32)[None, :], (128, 1)).copy()

    maps = []
    for core in range(cfg.NC):
        m = {"attT": attT, "basisf": basisf, "fcrT": fcrT, "fcb2": fcb2, "ic": IC}
        s0 = host_prep_streams(cfg, drug_feat, cj_drug, ci_dis, src, dst, core)
        s1 = host_prep_streams(cfg, dis_feat, cj_dis, ci_drug, dst, src, core)
        for k in range(NK):
            m[f"d0g{k}"], m[f"d0sl{k}"], m[f"d0sc{k}"] = s0[f"g{k}"], s0[f"sl{k}"], s0[f"sc{k}"]
            m[f"d1g{k}"], m[f"d1sl{k}"], m[f"d1sc{k}"] = s1[f"g{k}"], s1[f"sl{k}"], s1[f"sc{k}"]
        maps.append(m)
    return maps


def assemble_output(cfg, results):
    dis_out = np.concatenate([results[c]["outT"][0].T for c in range(cfg.NC)], 0)
    drug_out = np.concatenate([results[c]["outT"][1].T for c in range(cfg.NC)], 0)
    return drug_out, dis_out


def build_kernel(cfg):
    nc = bacc.Bacc(None, target_bir_lowering=False, debug=True)
    IN, NCALL, NWP = cfg.IN, cfg.NCALL, cfg.NWP
    NPAGE, WPP, SH, PPR = cfg.NPAGE, cfg.WPP, cfg.SH, cfg.PPR
    CPC, WPC, NCHUNK = cfg.CPC, cfg.WPC, cfg.NCHUNK
    NSLOT, DCH, NDC, SPS, NSTG = cfg.NSLOT, cfg.DCH, cfg.NDC, cfg.SPS, cfg.NSTG
    WROUND = cfg.WROUND
    PASSES = [(d, k) for d in range(2) for k in range(NK)]

    attT_d = nc.declare_dram_parameter("attT", [4, R], F32, isOutput=False)
    basisf_d = nc.declare_dram_parameter("basisf", [4, IN * MU], F32, isOutput=False)
    fcr_d = nc.declare_dram_parameter("fcrT", [MU, R * NK, OUT], F32, isOutput=False)
    fcb_d = nc.declare_dram_parameter("fcb2", [128, 2], F32, isOutput=False)
    ic_d = nc.declare_dram_parameter("ic", [128, 128], F32, isOutput=False)
    gD, slD, scD = {}, {}, {}
    for d, k in PASSES:
        gD[d, k] = nc.declare_dram_parameter(f"d{d}g{k}", [128, NCALL * 64], I16, isOutput=False)
        slD[d, k] = nc.declare_dram_parameter(f"d{d}sl{k}", [128, NWP], F32, isOutput=False)
        scD[d, k] = nc.declare_dram_parameter(f"d{d}sc{k}", [128, NWP], F32, isOutput=False)
    outT_d = nc.declare_dram_parameter("outT", [2, OUT, SH], F32, isOutput=True)

    wtab = nc.dram_tensor("wtab", [R, IN * MU], F32)
    wtab_rows = wtab[:].rearrange("r (f m) -> (r f) m", m=MU)
    hT = nc.dram_tensor("hT", [2, NK, MU, NSLOT], F32)

    attT_sb = nc.alloc_sbuf_tensor("attT_sb", [4, R], F32)
    bchunk = nc.alloc_sbuf_tensor("bchunk", [4, 2048], F32)
    wstage = nc.alloc_sbuf_tensor("wstage", [R, 2048], F32)
    ic_sb = nc.alloc_sbuf_tensor("ic_sb", [128, 128], F32)
    fcr_sb = nc.alloc_sbuf_tensor("fcr_sb", [MU, R * NK, OUT], F32)
    fcb_sb = nc.alloc_sbuf_tensor("fcb_sb", [128, 2], F32)
    NIB = 2
    gbuf = nc.alloc_sbuf_tensor("gbuf", [128, NIB, CPC * 64], I16)
    slbuf = nc.alloc_sbuf_tensor("slbuf", [128, NIB, WPC], F32)
    scbuf = nc.alloc_sbuf_tensor("scbuf", [128, NIB, WPC], F32)
    NMB = 4
    msgs = [nc.alloc_sbuf_tensor(f"msgs{i}", [128, 8, MU], F32) for i in range(NMB)]
    NSB = 4
    segt = [nc.alloc_sbuf_tensor(f"segt{i}", [128, 128], F32) for i in range(NSB)]
    NSTB = 2
    stage = [nc.alloc_sbuf_tensor(f"stage{i}", [MU, SPS * 128], F32) for i in range(NSTB)]
    prhs = [nc.alloc_sbuf_tensor(f"prhs{i}", [MU, R * NK, DCH], F32) for i in range(2)]
    ostage = [nc.alloc_sbuf_tensor(f"ostage{i}", [128, DCH], F32) for i in range(4)]

    NPB = 4
    psA = nc.alloc_psum_tensor("psA", [128, 2048], F32)
    pages = [psA[0:MU, i * 512:i * 512 + 128] for i in range(NPB)]
    psB = nc.alloc_psum_tensor("psB", [128, 2048], F32)
    wps = psB[0:R, :]
    pps = [psB[:, j * 512:(j + 1) * 512] for j in range(4)]  # j = 2*(l%2)+h

    def page_of_window(w):
        return min(w // WPP, NPAGE - 1)

    wlast = {}
    for pi in range(len(PASSES)):
        for w in range(NWP):
            wlast[pi, page_of_window(w)] = pi * NWP + w

    def chunk_cols(ch):
        c0 = ch * CPC
        return c0, min(CPC, NCALL - c0)

    # global chunk -> (global calls through chunk, global windows through chunk)
    chk_calls, chk_wins = [], []
    tc = tw = 0
    for pi in range(len(PASSES)):
        for ch in range(NCHUNK):
            c0, ncc = chunk_cols(ch)
            tc += ncc
            tw += ncc * 8
            chk_calls.append(tc)
            chk_wins.append(tw)

    sems = {}

    class S:
        pass

    with nc.Block() as block:
        for name, n in [("gth", NMB), ("inb", NIB), ("stg", NSTB),
                        ("pin", 2), ("ost", 4)]:
            for i in range(n):
                sems[name, i] = nc.alloc_semaphore(f"s_{name}{i}")
        for name in ["wdma", "wout", "wmm", "wcp", "seg", "pe", "act", "pmm", "oact"]:
            sems[name] = nc.alloc_semaphore(f"s_{name}")
        s_gth = [sems["gth", i] for i in range(NMB)]
        s_inb = [sems["inb", i] for i in range(NIB)]
        s_stg = [sems["stg", i] for i in range(NSTB)]
        s_pin = [sems["pin", i] for i in range(2)]
        s_ost = [sems["ost", i] for i in range(4)]
        s_wdma, s_wmm, s_wcp = sems["wdma"], sems["wmm"], sems["wcp"]
        s_wout = sems["wout"]
        s_seg, s_pe, s_act = sems["seg"], sems["pe"], sems["act"]
        s_pmm, s_oact = sems["pmm"], sems["oact"]

        # ============ GPSIMD: const + W-build DMAs, then gathers
        @block.gpsimd
        def _(g):
            g.dma_start(attT_sb[:], attT_d[:]).then_inc(s_wdma, 16)
            g.dma_start(ic_sb[:], ic_d[:]).then_inc(s_wdma, 16)
            g.dma_start(fcr_sb[:], fcr_d[:]).then_inc(s_wdma, 16)
            g.dma_start(fcb_sb[:], fcb_d[:]).then_inc(s_wdma, 16)
            for n in range(WROUND):
                g.wait_ge(s_wcp, n)  # round n-1 psum copied (wstage free after out-DMA below)
                g.dma_start(bchunk[:], basisf_d[:, n*2048:(n+1)*2048]).then_inc(s_wdma, 16)
                g.wait_ge(s_wcp, n + 1)
                g.dma_start(wtab[:, n*2048:(n+1)*2048], wstage[:]).then_inc(s_wout, 16)
            g.wait_ge(s_wout, WROUND * 16)  # all wtab writes landed
            ncall = 0
            for pi, (d, k) in enumerate(PASSES):
                for c in range(NCALL):
                    gch = pi * NCHUNK + c // CPC
                    g.wait_ge(s_inb[gch % NIB], 48 * (gch // NIB + 1))
                    b = ncall % NMB
                    if ncall >= NMB:
                        g.wait_ge(s_pe, 8 * (ncall - NMB + 1))
                    g.dma_gather(
                        msgs[b][:], wtab_rows,
                        gbuf[:, gch % NIB, (c % CPC) * 64:(c % CPC + 1) * 64],
                        1024, 1024, MU,
                    ).then_inc(s_gth[b], 16)
                    ncall += 1

        # ============ TENSOR: W MMs, window MMs, projection MMs
        @block.tensor
        def _(t):
            for n in range(WROUND):
                t.wait_ge(s_wdma, 64 + 16 * (n + 1))
                if n >= 1:
                    t.wait_ge(s_wcp, n)
                for i in range(4):
                    ins = t.matmul(wps[:, i*512:(i+1)*512], attT_sb[:],
                                   bchunk[:, i*512:(i+1)*512],
                                   start=True, stop=True)
                ins.then_inc(s_wmm, 1)
            wglob = 0
            for pi, (d, k) in enumerate(PASSES):
                for w in range(NWP):
                    p = page_of_window(w)
                    pglob = pi * NPAGE + p
                    first = (w % WPP == 0) and (p == w // WPP)
                    if first and pglob >= NPB:
                        t.wait_ge(s_act, pglob - NPB + 1)
                    b = (wglob // 8) % NMB
                    t.wait_ge(s_gth[b], 16 * (wglob // 8 // NMB + 1))
                    t.wait_ge(s_seg, wglob + 1)
                    t.matmul(pages[pglob % NPB],
                             msgs[b][:, w % 8, :],
                             segt[wglob % NSB][:],
                             start=first, stop=(wglob == wlast[pi, p]),
                             ).then_inc(s_pe, 1)
                    wglob += 1
            nl = 0
            for d in range(2):
                for c in range(NDC):
                    ncols = min(DCH, SH - c * DCH)
                    t.wait_ge(s_pin[nl % 2], 240 * (nl // 2 + 1))
                    if nl >= 2:
                        t.wait_ge(s_oact, 2 * (nl - 1))
                    for h in range(2):
                        for rk in range(R * NK):
                            ins = t.matmul(pps[2*(nl % 2)+h][:, :ncols],
                                           fcr_sb[:, rk, h*128:(h+1)*128],
                                           prhs[nl % 2][:, rk, :ncols],
                                           start=(rk == 0), stop=(rk == R*NK-1))
                        ins.then_inc(s_pmm, 1)
                    nl += 1

        # ============ VECTOR: W psum->sbuf copies, SegT builds
        @block.vector
        def _(v):
            for n in range(WROUND):
                v.wait_ge(s_wmm, n + 1)
                if n >= 1:
                    v.wait_ge(s_wout, 16 * n)
                v.tensor_copy(wstage[:], wps[:]).then_inc(s_wcp, 1)
            wglob = 0
            for pi, (d, k) in enumerate(PASSES):
                for w in range(NWP):
                    gch = pi * NCHUNK + (w // 8) // CPC
                    v.wait_ge(s_inb[gch % NIB], 48 * (gch // NIB + 1))
                    if wglob >= NSB:
                        v.wait_ge(s_pe, wglob - NSB + 1)
                    wc = w % WPC
                    v.tensor_scalar(
                        segt[wglob % NSB][:], ic_sb[:],
                        slbuf[:, gch % NIB, wc:wc+1],
                        scbuf[:, gch % NIB, wc:wc+1],
                        mybir.AluOpType.is_equal, mybir.AluOpType.mult,
                    ).then_inc(s_seg, 1)
                    wglob += 1
            nl = 0
            for d in range(2):
                for c in range(NDC):
                    ncols = min(DCH, SH - c * DCH)
                    for h in range(2):
                        ob = 2 * (nl % 2) + h
                        v.wait_ge(s_pmm, 2 * nl + h + 1)
                        if nl >= 2:
                            v.wait_ge(s_ost[ob], 16 * (nl // 2))
                        v.tensor_scalar(
                            ostage[ob][:, :ncols], pps[ob][:, :ncols],
                            fcb_sb[:, h:h+1], None,
                            mybir.AluOpType.add,
                        ).then_inc(s_oact, 1)
                    nl += 1

        # ============ SCALAR: page->stage copies; projection psum->ostage
        @block.scalar
        def _(a):
            pglob = 0
            for pi, (d, k) in enumerate(PASSES):
                for p in range(NPAGE):
                    st = p // SPS
                    stglob = pi * NSTG + st
                    a.wait_ge(s_pe, wlast[pi, p] + 1)
                    if stglob >= NSTB and p % SPS == 0:
                        a.wait_ge(s_stg[stglob % NSTB], 16 * (stglob // NSTB))
                    a.copy(stage[stglob % NSTB][:, (p % SPS)*128:(p % SPS+1)*128],
                           pages[pglob % NPB]).then_inc(s_act, 1)
                    pglob += 1
                    if p % SPS == SPS - 1 or p == NPAGE - 1:
                        p0 = st * SPS
                        npg = p - p0 + 1
                        a.wait_ge(s_act, pglob)
                        a.dma_start(hT[d, k][:, p0*128:(p0+npg)*128],
                                    stage[stglob % NSTB][:, :npg*128]
                                    ).then_inc(s_stg[stglob % NSTB], 16)

        # ============ SYNC: input chunks, stage->hT, proj loads, out DMAs
        @block.sync
        def _(s):
            gch = 0
            for pi, (d, k) in enumerate(PASSES):
                for ch in range(NCHUNK):
                    if gch >= NIB:
                        s.wait_ge(s_pe, chk_wins[gch - NIB])
                    ib = gch % NIB
                    c0, ncc = chunk_cols(ch)
                    s.dma_start(gbuf[:, ib, :ncc*64], gD[d, k][:, c0*64:(c0+ncc)*64]).then_inc(s_inb[ib], 16)
                    s.dma_start(slbuf[:, ib, :ncc*8], slD[d, k][:, c0*8:(c0+ncc)*8]).then_inc(s_inb[ib], 16)
                    s.dma_start(scbuf[:, ib, :ncc*8], scD[d, k][:, c0*8:(c0+ncc)*8]).then_inc(s_inb[ib], 16)
                    gch += 1
            # wait all stage->hT DMAs before projection loads
            NSTGALL = len(PASSES) * NSTG
            for b in range(NSTB):
                occ = (NSTGALL - b + NSTB - 1) // NSTB
                s.wait_ge(s_stg[b], 16 * occ)
            nl = 0
            for d in range(2):
                for c in range(NDC):
                    ncols = min(DCH, SH - c * DCH)
                    if nl >= 2:
                        s.wait_ge(s_pmm, 2 * (nl - 1))
                    for rk in range(R * NK):
                        r, k = rk // NK, rk % NK
                        s.dma_start(
                            prhs[nl % 2][:, rk, :ncols],
                            hT[d, k][:, r*PPR*128 + c*DCH: r*PPR*128 + c*DCH + ncols]
                        ).then_inc(s_pin[nl % 2], 16)
                    for h in range(2):
                        ob = 2 * (nl % 2) + h
                        s.wait_ge(s_oact, 2 * nl + h + 1)
                        s.dma_start(outT_d[d, h*128:(h+1)*128, c*DCH:c*DCH+ncols],
                                    ostage[ob][:, :ncols]).then_inc(s_ost[ob], 16)
                    nl += 1
            NLD = 2 * NDC
            for b in range(2):
                occ = (NLD - b + 1) // 2
                for h in range(2):
                    s.wait_ge(s_ost[2 * b + h], 16 * occ)

    nc.compile()
    return nc


# ======================================================================
# Self-contained kernel entry point.
# ======================================================================
from concourse.bass_utils import run_bass_kernel_spmd as _run_spmd

_CACHE = {}


def kernel(**inputs):
    """GCMC layer on 8 trn2 NeuronCores. Returns (drug_out, dis_out) f32."""
    cfg = Cfg(50000, 1024, 8, wpp=12)
    maps = build_inputs(cfg, inputs)
    if "nc" not in _CACHE:
        _CACHE["nc"] = build_kernel(cfg)
    res = _run_spmd(_CACHE["nc"], maps, list(range(cfg.NC)))
    return assemble_output(cfg, res.results)



# revision 2
# speedup vs baseline: 3.4626x; 3.4626x over previous
"""GCMC message-passing kernel for trn2: builder + host preprocessing.

Per core = one dst-shard, both directions (0: drug->dis, 1: dis->drug).
  Phase W: device computes W[r] = att @ basis -> wtab[R, IN*MU] f32 in HBM.
  Phase E (x6 passes = 2 dirs x 3 k-feats): per-edge event streams sorted by
    slot (r-major, dst-local), 128-event windows, WPP windows per 128-slot
    page. dma_gather pulls 64-f32 W rows (1024 events/call); DVE builds
    SegT[128ev,128slot] = is_equal(IC, sl) * sc  (sc = cj*ci, host-folded);
    PE: msgs.T @ SegT accumulated into a PSUM page [MU, 128].
    Pages -> SBUF stage (ACT, bf16) -> hT[d,k] = [MU, NSLOT] bf16 HBM (SYNC).
  Phase P: outT[d] [256, SH] = sum_rk fcblk_rk.T @ hT-slices + bias (f16 out).

Wire-format choices (the axon tunnel is ~50MB/s, so bytes dominate wall):
  gD[d,k]  int16 [16, NCALL*64]  -- un-replicated; device copies it into all
                                    eight 16-partition groups of the SBUF
                                    index buffer (dma_gather wants 8 replicas).
  slD[d]   uint8 [128, NWP]      -- slot-in-page, shared across the 3 k-passes.
  scD[d]   bf16  [128, NWP]      -- cj*ci edge scale, shared across k-passes.
  fcrT     bf16, hT staging bf16, outT float16.
Host assembles + transposes the two outputs (float32).
"""
import numpy as np
import ml_dtypes
import concourse.bass as bass
import concourse.bacc as bacc
import concourse.mybir as mybir

F32 = mybir.dt.float32
F16 = mybir.dt.float16
BF16 = mybir.dt.bfloat16
I16 = mybir.dt.int16
U8 = mybir.dt.uint8

NP_BF16 = ml_dtypes.bfloat16

R = 5
MU = 64
OUT = 256
NK = 3


class Cfg:
    def __init__(self, n_nodes, in_units, n_cores, wpp):
        self.N = n_nodes
        self.IN = in_units
        self.NC = n_cores
        self.SH = n_nodes // n_cores
        self.PPR = (self.SH + 127) // 128
        self.NPAGE = R * self.PPR
        self.NSLOT = self.NPAGE * 128
        self.WPP = wpp
        self.NW = self.NPAGE * wpp
        self.NWP = ((self.NW + 7) // 8) * 8
        self.NCALL = self.NWP // 8
        self.DCH = 512
        self.NDC = (self.SH + self.DCH - 1) // self.DCH
        self.SPS = 16
        self.NSTG = (self.NPAGE + self.SPS - 1) // self.SPS
        self.WROUND = (in_units * MU) // 2048
        assert (in_units * MU) % 2048 == 0


def host_prep_streams(cfg, feat_src, cj_src, ci_dst, src, dst, core):
    SH, PPR, WPP = cfg.SH, cfg.PPR, cfg.WPP
    lo, hi = core * SH, (core + 1) * SH
    evg = [[] for _ in range(NK)]
    evsl, evsc = [], []
    for r in range(R):
        m = (dst[r] >= lo) & (dst[r] < hi)
        s, d = src[r][m], dst[r][m]
        slot = r * PPR * 128 + (d - lo)
        sc = (cj_src[s, 0] * ci_dst[d, 0]).astype(np.float32)
        for k in range(NK):
            evg[k].append((r * cfg.IN + feat_src[s, k]).astype(np.int64))
        evsl.append(slot.astype(np.int64))
        evsc.append(sc)
    sl = np.concatenate(evsl)
    sc = np.concatenate(evsc)
    order = np.argsort(sl, kind="stable")
    sl, sc = sl[order], sc[order]
    NWP = cfg.NWP
    SL = np.zeros((NWP, 128), np.uint8)
    SC = np.zeros((NWP, 128), np.float32)
    page = sl // 128
    counts = np.bincount(page, minlength=cfg.NPAGE)
    assert counts.max() <= WPP * 128, (
        f"page overflow {counts.max()} > {WPP*128}; raise WPP")
    # window boundaries, shared by all three k streams
    bounds = []
    pos = 0
    for p in range(cfg.NPAGE):
        n = counts[p]
        sls, scs = sl[pos:pos+n], sc[pos:pos+n]
        for w in range((n + 127) // 128):
            a, b = w * 128, min((w + 1) * 128, n)
            wi = p * WPP + w
            SL[wi, :b-a] = (sls[a:b] - p * 128).astype(np.uint8)
            SC[wi, :b-a] = scs[a:b]
            bounds.append((wi, pos + a, pos + b))
        pos += n
    out = {
        "sl": SL.T.copy(),
        "sc": SC.T.astype(NP_BF16).copy(),
    }
    for k in range(NK):
        g = np.concatenate(evg[k])[order]
        G = np.zeros((NWP, 128), np.int16)
        for wi, a, b in bounds:
            G[wi, :b-a] = g[a:b]
        Gc = G.reshape(cfg.NCALL, 1024)
        wr = np.zeros((16, cfg.NCALL * 64), np.int16)
        for c in range(cfg.NCALL):
            wr[:, c*64:(c+1)*64] = Gc[c].reshape(64, 16).T
        out[f"g{k}"] = wr
    return out


def build_inputs(cfg, inputs):
    f32 = np.float32
    gi = lambda n: np.asarray(inputs[n], np.int64)
    gf = lambda n: np.asarray(inputs[n], f32)
    drug_feat, dis_feat = gi("drug_feat"), gi("dis_feat")
    src, dst = gi("src"), gi("dst")
    cj_drug, ci_drug = gf("cj_drug"), gf("ci_drug")
    cj_dis, ci_dis = gf("cj_dis"), gf("ci_dis")
    att, basis = gf("att"), gf("basis")
    fc_w, fc_b = gf("fc_w"), gf("fc_b")

    attT = att.T.copy()
    basisf = basis.reshape(4, cfg.IN * MU).copy()
    # fcrT[m, rk, o] = fc_w[r*NK*MU + k*MU + m, o]
    fcrT = fc_w.reshape(R * NK, MU, OUT).transpose(1, 0, 2).astype(NP_BF16).copy()
    fcb2 = fc_b.reshape(2, 128).T.copy()      # [128, 2], col h = half h
    IC = np.tile(np.arange(128, dtype=f32)[None, :], (128, 1)).copy()

    maps = []
    for core in range(cfg.NC):
        m = {"attT": attT, "basisf": basisf, "fcrT": fcrT, "fcb2": fcb2, "ic": IC}
        s0 = host_prep_streams(cfg, drug_feat, cj_drug, ci_dis, src, dst, core)
        s1 = host_prep_streams(cfg, dis_feat, cj_dis, ci_drug, dst, src, core)
        for d, s in ((0, s0), (1, s1)):
            m[f"d{d}sl"], m[f"d{d}sc"] = s["sl"], s["sc"]
            for k in range(NK):
                m[f"d{d}g{k}"] = s[f"g{k}"]
        maps.append(m)
    return maps


def assemble_output(cfg, results):
    dis_out = np.concatenate(
        [results[c]["outT"][0].T.astype(np.float32) for c in range(cfg.NC)], 0)
    drug_out = np.concatenate(
        [results[c]["outT"][1].T.astype(np.float32) for c in range(cfg.NC)], 0)
    return drug_out, dis_out


def build_kernel(cfg):
    nc = bacc.Bacc(None, target_bir_lowering=False, debug=True)
    IN, NCALL, NWP = cfg.IN, cfg.NCALL, cfg.NWP
    NPAGE, WPP, SH, PPR = cfg.NPAGE, cfg.WPP, cfg.SH, cfg.PPR
    NSLOT, DCH, NDC, SPS, NSTG = cfg.NSLOT, cfg.DCH, cfg.NDC, cfg.SPS, cfg.NSTG
    WROUND = cfg.WROUND
    PASSES = [(d, k) for d in range(2) for k in range(NK)]

    attT_d = nc.declare_dram_parameter("attT", [4, R], F32, isOutput=False)
    basisf_d = nc.declare_dram_parameter("basisf", [4, IN * MU], F32, isOutput=False)
    fcr_d = nc.declare_dram_parameter("fcrT", [MU, R * NK, OUT], BF16, isOutput=False)
    fcb_d = nc.declare_dram_parameter("fcb2", [128, 2], F32, isOutput=False)
    ic_d = nc.declare_dram_parameter("ic", [128, 128], F32, isOutput=False)
    gD, slD, scD = {}, {}, {}
    for d, k in PASSES:
        gD[d, k] = nc.declare_dram_parameter(f"d{d}g{k}", [16, NCALL * 64], I16, isOutput=False)
    for d in range(2):
        slD[d] = nc.declare_dram_parameter(f"d{d}sl", [128, NWP], U8, isOutput=False)
        scD[d] = nc.declare_dram_parameter(f"d{d}sc", [128, NWP], BF16, isOutput=False)
    outT_d = nc.declare_dram_parameter("outT", [2, OUT, SH], F16, isOutput=True)

    wtab = nc.dram_tensor("wtab", [R, IN * MU], F32)
    wtab_rows = wtab[:].rearrange("r (f m) -> (r f) m", m=MU)
    hT = nc.dram_tensor("hT", [2, NK, MU, NSLOT], BF16)

    attT_sb = nc.alloc_sbuf_tensor("attT_sb", [4, R], F32)
    bchunk = nc.alloc_sbuf_tensor("bchunk", [4, 2048], F32)
    wstage = nc.alloc_sbuf_tensor("wstage", [R, 2048], F32)
    ic_sb = nc.alloc_sbuf_tensor("ic_sb", [128, 128], F32)
    fcr_sb = nc.alloc_sbuf_tensor("fcr_sb", [MU, R * NK, OUT], BF16)
    fcb_sb = nc.alloc_sbuf_tensor("fcb_sb", [128, 2], F32)
    gsb = nc.alloc_sbuf_tensor("gsb", [128, NCALL * 64], I16)
    slr = nc.alloc_sbuf_tensor("slr", [128, NWP], U8)
    scr = nc.alloc_sbuf_tensor("scr", [128, NWP], BF16)
    slf = nc.alloc_sbuf_tensor("slf", [128, NWP], F32)
    scf = nc.alloc_sbuf_tensor("scf", [128, NWP], F32)
    NMB = 4
    msgs = [nc.alloc_sbuf_tensor(f"msgs{i}", [128, 8, MU], F32) for i in range(NMB)]
    NSB = 4
    segt = [nc.alloc_sbuf_tensor(f"segt{i}", [128, 128], F32) for i in range(NSB)]
    NSTB = 2
    stage = [nc.alloc_sbuf_tensor(f"stage{i}", [MU, SPS * 128], BF16) for i in range(NSTB)]
    prhs = [nc.alloc_sbuf_tensor(f"prhs{i}", [MU, R * NK, DCH], BF16) for i in range(2)]
    ostage = [nc.alloc_sbuf_tensor(f"ostage{i}", [128, DCH], F16) for i in range(4)]

    NPB = 4
    psA = nc.alloc_psum_tensor("psA", [128, 2048], F32)
    pages = [psA[0:MU, i * 512:i * 512 + 128] for i in range(NPB)]
    psB = nc.alloc_psum_tensor("psB", [128, 2048], F32)
    wps = psB[0:R, :]
    pps = [psB[:, j * 512:(j + 1) * 512] for j in range(4)]  # j = 2*(l%2)+h

    def page_of_window(w):
        return min(w // WPP, NPAGE - 1)

    wlast = {}
    for pi in range(len(PASSES)):
        for w in range(NWP):
            wlast[pi, page_of_window(w)] = pi * NWP + w

    sems = {}

    with nc.Block() as block:
        for name, n in [("gth", NMB), ("stg", NSTB), ("pin", 2), ("ost", 4)]:
            for i in range(n):
                sems[name, i] = nc.alloc_semaphore(f"s_{name}{i}")
        for name in ["wdma", "wout", "wmm", "wcp", "seg", "pe", "act", "pmm",
                     "oact", "gsb", "slraw"]:
            sems[name] = nc.alloc_semaphore(f"s_{name}")
        s_gth = [sems["gth", i] for i in range(NMB)]
        s_stg = [sems["stg", i] for i in range(NSTB)]
        s_pin = [sems["pin", i] for i in range(2)]
        s_ost = [sems["ost", i] for i in range(4)]
        s_wdma, s_wmm, s_wcp = sems["wdma"], sems["wmm"], sems["wcp"]
        s_wout = sems["wout"]
        s_seg, s_pe, s_act = sems["seg"], sems["pe"], sems["act"]
        s_pmm, s_oact = sems["pmm"], sems["oact"]
        s_gsb, s_slraw = sems["gsb"], sems["slraw"]

        # ============ GPSIMD: const + W-build DMAs, then gathers
        @block.gpsimd
        def _(g):
            g.dma_start(attT_sb[:], attT_d[:]).then_inc(s_wdma, 16)
            g.dma_start(ic_sb[:], ic_d[:]).then_inc(s_wdma, 16)
            g.dma_start(fcr_sb[:], fcr_d[:]).then_inc(s_wdma, 16)
            g.dma_start(fcb_sb[:], fcb_d[:]).then_inc(s_wdma, 16)
            for n in range(WROUND):
                g.wait_ge(s_wcp, n)  # round n-1 psum copied (wstage free after out-DMA below)
                g.dma_start(bchunk[:], basisf_d[:, n*2048:(n+1)*2048]).then_inc(s_wdma, 16)
                g.wait_ge(s_wcp, n + 1)
                g.dma_start(wtab[:, n*2048:(n+1)*2048], wstage[:]).then_inc(s_wout, 16)
            g.wait_ge(s_wout, WROUND * 16)  # all wtab writes landed
            ncall = 0
            for pi, (d, k) in enumerate(PASSES):
                g.wait_ge(s_gsb, 128 * (pi + 1))  # this pass's gsb loaded
                for c in range(NCALL):
                    b = ncall % NMB
                    if ncall >= NMB:
                        g.wait_ge(s_pe, 8 * (ncall - NMB + 1))
                    g.dma_gather(
                        msgs[b][:], wtab_rows,
                        gsb[:, c * 64:(c + 1) * 64],
                        1024, 1024, MU,
                    ).then_inc(s_gth[b], 16)
                    ncall += 1

        # ============ TENSOR: W MMs, window MMs, projection MMs
        @block.tensor
        def _(t):
            for n in range(WROUND):
                t.wait_ge(s_wdma, 64 + 16 * (n + 1))
                if n >= 1:
                    t.wait_ge(s_wcp, n)
                for i in range(4):
                    ins = t.matmul(wps[:, i*512:(i+1)*512], attT_sb[:],
                                   bchunk[:, i*512:(i+1)*512],
                                   start=True, stop=True)
                ins.then_inc(s_wmm, 1)
            wglob = 0
            for pi, (d, k) in enumerate(PASSES):
                for w in range(NWP):
                    p = page_of_window(w)
                    pglob = pi * NPAGE + p
                    first = (w % WPP == 0) and (p == w // WPP)
                    if first and pglob >= NPB:
                        t.wait_ge(s_act, pglob - NPB + 1)
                    b = (wglob // 8) % NMB
                    t.wait_ge(s_gth[b], 16 * (wglob // 8 // NMB + 1))
                    t.wait_ge(s_seg, wglob + 1)
                    t.matmul(pages[pglob % NPB],
                             msgs[b][:, w % 8, :],
                             segt[wglob % NSB][:],
                             start=first, stop=(wglob == wlast[pi, p]),
                             ).then_inc(s_pe, 1)
                    wglob += 1
            nl = 0
            for d in range(2):
                for c in range(NDC):
                    ncols = min(DCH, SH - c * DCH)
                    t.wait_ge(s_pin[nl % 2], 240 * (nl // 2 + 1))
                    if nl >= 2:
                        t.wait_ge(s_oact, 2 * (nl - 1))
                    for h in range(2):
                        for rk in range(R * NK):
                            ins = t.matmul(pps[2*(nl % 2)+h][:, :ncols],
                                           fcr_sb[:, rk, h*128:(h+1)*128],
                                           prhs[nl % 2][:, rk, :ncols],
                                           start=(rk == 0), stop=(rk == R*NK-1))
                        ins.then_inc(s_pmm, 1)
                    nl += 1

        # ============ VECTOR: W psum->sbuf copies, sl/sc converts, SegT builds
        @block.vector
        def _(v):
            for n in range(WROUND):
                v.wait_ge(s_wmm, n + 1)
                if n >= 1:
                    v.wait_ge(s_wout, 16 * n)
                v.tensor_copy(wstage[:], wps[:]).then_inc(s_wcp, 1)
            wglob = 0
            for pi, (d, k) in enumerate(PASSES):
                if k == 0:
                    # direction start: widen sl u8 / sc bf16 to f32 once
                    v.wait_ge(s_slraw, 32 * (d + 1))
                    v.tensor_copy(slf[:], slr[:])
                    v.tensor_copy(scf[:], scr[:])
                for w in range(NWP):
                    if wglob >= NSB:
                        v.wait_ge(s_pe, wglob - NSB + 1)
                    v.tensor_scalar(
                        segt[wglob % NSB][:], ic_sb[:],
                        slf[:, w:w+1],
                        scf[:, w:w+1],
                        mybir.AluOpType.is_equal, mybir.AluOpType.mult,
                    ).then_inc(s_seg, 1)
                    wglob += 1
            nl = 0
            for d in range(2):
                for c in range(NDC):
                    ncols = min(DCH, SH - c * DCH)
                    for h in range(2):
                        ob = 2 * (nl % 2) + h
                        v.wait_ge(s_pmm, 2 * nl + h + 1)
                        if nl >= 2:
                            v.wait_ge(s_ost[ob], 16 * (nl // 2))
                        v.tensor_scalar(
                            ostage[ob][:, :ncols], pps[ob][:, :ncols],
                            fcb_sb[:, h:h+1], None,
                            mybir.AluOpType.add,
                        ).then_inc(s_oact, 1)
                    nl += 1

        # ============ SCALAR: page->stage copies (bf16); stage->hT DMAs
        @block.scalar
        def _(a):
            pglob = 0
            for pi, (d, k) in enumerate(PASSES):
                for p in range(NPAGE):
                    st = p // SPS
                    stglob = pi * NSTG + st
                    a.wait_ge(s_pe, wlast[pi, p] + 1)
                    if stglob >= NSTB and p % SPS == 0:
                        a.wait_ge(s_stg[stglob % NSTB], 16 * (stglob // NSTB))
                    a.copy(stage[stglob % NSTB][:, (p % SPS)*128:(p % SPS+1)*128],
                           pages[pglob % NPB]).then_inc(s_act, 1)
                    pglob += 1
                    if p % SPS == SPS - 1 or p == NPAGE - 1:
                        p0 = st * SPS
                        npg = p - p0 + 1
                        a.wait_ge(s_act, pglob)
                        a.dma_start(hT[d, k][:, p0*128:(p0+npg)*128],
                                    stage[stglob % NSTB][:, :npg*128]
                                    ).then_inc(s_stg[stglob % NSTB], 16)

        # ============ SYNC: g replication + sl/sc loads, proj loads, out DMAs
        @block.sync
        def _(s):
            for pi, (d, k) in enumerate(PASSES):
                if pi >= 1:
                    s.wait_ge(s_pe, pi * NWP)   # gsb (and for k==0, slr/scr) free
                if k == 0:
                    s.dma_start(slr[:], slD[d][:]).then_inc(s_slraw, 16)
                    s.dma_start(scr[:], scD[d][:]).then_inc(s_slraw, 16)
                for rep in range(8):
                    s.dma_start(gsb[rep*16:(rep+1)*16, :], gD[d, k][:]
                                ).then_inc(s_gsb, 16)
            # wait all stage->hT DMAs before projection loads
            NSTGALL = len(PASSES) * NSTG
            for b in range(NSTB):
                occ = (NSTGALL - b + NSTB - 1) // NSTB
                s.wait_ge(s_stg[b], 16 * occ)
            nl = 0
            for d in range(2):
                for c in range(NDC):
                    ncols = min(DCH, SH - c * DCH)
                    if nl >= 2:
                        s.wait_ge(s_pmm, 2 * (nl - 1))
                    for rk in range(R * NK):
                        r, k = rk // NK, rk % NK
                        s.dma_start(
                            prhs[nl % 2][:, rk, :ncols],
                            hT[d, k][:, r*PPR*128 + c*DCH: r*PPR*128 + c*DCH + ncols]
                        ).then_inc(s_pin[nl % 2], 16)
                    for h in range(2):
                        ob = 2 * (nl % 2) + h
                        s.wait_ge(s_oact, 2 * nl + h + 1)
                        s.dma_start(outT_d[d, h*128:(h+1)*128, c*DCH:c*DCH+ncols],
                                    ostage[ob][:, :ncols]).then_inc(s_ost[ob], 16)
                    nl += 1
            NLD = 2 * NDC
            for b in range(2):
                occ = (NLD - b + 1) // 2
                for h in range(2):
                    s.wait_ge(s_ost[2 * b + h], 16 * occ)

    nc.compile()
    return nc


# ======================================================================
# Self-contained kernel entry point.
# ======================================================================
from concourse.bass_utils import run_bass_kernel_spmd as _run_spmd

_CACHE = {}


def kernel(**inputs):
    """GCMC layer on 8 trn2 NeuronCores. Returns (drug_out, dis_out) f32."""
    cfg = Cfg(50000, 1024, 8, wpp=12)
    maps = build_inputs(cfg, inputs)
    if "nc" not in _CACHE:
        _CACHE["nc"] = build_kernel(cfg)
    res = _run_spmd(_CACHE["nc"], maps, list(range(cfg.NC)))
    return assemble_output(cfg, res.results)


# revision 25
# speedup vs baseline: 4.4587x; 1.2877x over previous
"""GCMC message-passing kernel for trn2: builder + host preprocessing.

Per core = one dst-shard, both directions (0: drug->dis, 1: dis->drug).
  Phase W: device computes W[r] = att @ basis -> wtab[R, IN*MU] f32 in HBM.
  Phase E (x6 passes = 2 dirs x 3 k-feats): per-edge event streams sorted by
    slot (r-major, dst-local), 128-event windows, WPP windows per 128-slot
    page. dma_gather pulls 64-f32 W rows (1024 events/call); DVE builds
    SegT[128ev,128slot] = is_equal(IC, sl) * sc  (sc = cj*ci, host-folded);
    PE: msgs.T @ SegT accumulated into a PSUM page [MU, 128].
    Pages -> SBUF stage (ACT, bf16) -> hT[d,k] = [MU, NSLOT] bf16 HBM (SYNC).
  Phase P: outT[d] [256, SH] = sum_rk fcblk_rk.T @ hT-slices + bias (f16 out).

Wire-format choices (the axon tunnel is ~50MB/s, so bytes dominate wall):
  gD[d,k]  int16 [16, NCALL*64]  -- un-replicated; device copies it into all
                                    eight 16-partition groups of the SBUF
                                    index buffer (dma_gather wants 8 replicas).
  slD[d]   uint8 [128, NWP]      -- slot-in-page, shared across the 3 k-passes.
  scD[d]   bf16  [128, NWP]      -- cj*ci edge scale, shared across k-passes.
  fcrT     bf16, hT staging bf16, outT float16.
Host assembles + transposes the two outputs (float32).
"""
import numpy as np
import ml_dtypes
import concourse.bass as bass
import concourse.bacc as bacc
import concourse.mybir as mybir

F32 = mybir.dt.float32
F16 = mybir.dt.float16
BF16 = mybir.dt.bfloat16
I16 = mybir.dt.int16
U8 = mybir.dt.uint8

NP_BF16 = ml_dtypes.bfloat16

R = 5
MU = 64
OUT = 256
NK = 3


class Cfg:
    def __init__(self, n_nodes, in_units, n_cores, wpp):
        self.N = n_nodes
        self.IN = in_units
        self.NC = n_cores
        self.SH = n_nodes // n_cores
        self.PPR = (self.SH + 127) // 128
        self.NPAGE = R * self.PPR
        self.NSLOT = self.NPAGE * 128
        self.WPP = wpp
        self.NW = self.NPAGE * wpp
        self.NWP = ((self.NW + 7) // 8) * 8
        self.NCALL = self.NWP // 8
        self.DCH = 512
        self.NDC = (self.SH + self.DCH - 1) // self.DCH
        self.SPS = 16
        self.NSTG = (self.NPAGE + self.SPS - 1) // self.SPS
        self.WROUND = (in_units * MU) // 2048
        assert (in_units * MU) % 2048 == 0


def host_prep_streams(cfg, feat_src, cj_src, ci_dst, src, dst, core):
    SH, PPR, WPP = cfg.SH, cfg.PPR, cfg.WPP
    lo, hi = core * SH, (core + 1) * SH
    evg = [[] for _ in range(NK)]
    evsl, evsc = [], []
    for r in range(R):
        m = (dst[r] >= lo) & (dst[r] < hi)
        s, d = src[r][m], dst[r][m]
        slot = r * PPR * 128 + (d - lo)
        sc = (cj_src[s, 0] * ci_dst[d, 0]).astype(np.float32)
        for k in range(NK):
            evg[k].append((r * cfg.IN + feat_src[s, k]).astype(np.int64))
        evsl.append(slot.astype(np.int64))
        evsc.append(sc)
    sl = np.concatenate(evsl)
    sc = np.concatenate(evsc)
    order = np.argsort(sl, kind="stable")
    sl, sc = sl[order], sc[order]
    NWP = cfg.NWP
    SL = np.zeros((NWP, 128), np.uint8)
    SC = np.zeros((NWP, 128), np.float32)
    page = sl // 128
    counts = np.bincount(page, minlength=cfg.NPAGE)
    assert counts.max() <= WPP * 128, (
        f"page overflow {counts.max()} > {WPP*128}; raise WPP")
    # window boundaries, shared by all three k streams
    bounds = []
    pos = 0
    for p in range(cfg.NPAGE):
        n = counts[p]
        sls, scs = sl[pos:pos+n], sc[pos:pos+n]
        for w in range((n + 127) // 128):
            a, b = w * 128, min((w + 1) * 128, n)
            wi = p * WPP + w
            SL[wi, :b-a] = (sls[a:b] - p * 128).astype(np.uint8)
            SC[wi, :b-a] = scs[a:b]
            bounds.append((wi, pos + a, pos + b))
        pos += n
    out = {
        "sl": SL.T.copy(),
        "sc": SC.T.astype(NP_BF16).copy(),
    }
    for k in range(NK):
        g = np.concatenate(evg[k])[order]
        G = np.zeros((NWP, 128), np.int16)
        for wi, a, b in bounds:
            G[wi, :b-a] = g[a:b]
        Gc = G.reshape(cfg.NCALL, 1024)
        wr = np.zeros((16, cfg.NCALL * 64), np.int16)
        for c in range(cfg.NCALL):
            wr[:, c*64:(c+1)*64] = Gc[c].reshape(64, 16).T
        out[f"g{k}"] = wr
    return out


def build_inputs(cfg, inputs):
    f32 = np.float32
    gi = lambda n: np.asarray(inputs[n], np.int64)
    gf = lambda n: np.asarray(inputs[n], f32)
    drug_feat, dis_feat = gi("drug_feat"), gi("dis_feat")
    src, dst = gi("src"), gi("dst")
    cj_drug, ci_drug = gf("cj_drug"), gf("ci_drug")
    cj_dis, ci_dis = gf("cj_dis"), gf("ci_dis")
    att, basis = gf("att"), gf("basis")
    fc_w, fc_b = gf("fc_w"), gf("fc_b")

    attT = att.T.copy()
    basisf = basis.reshape(4, cfg.IN * MU).copy()
    # fcrT[m, rk, o] = fc_w[r*NK*MU + k*MU + m, o]
    fcrT = fc_w.reshape(R * NK, MU, OUT).transpose(1, 0, 2).astype(NP_BF16).copy()
    fcb2 = fc_b.reshape(2, 128).T.copy()      # [128, 2], col h = half h
    IC = np.tile(np.arange(128, dtype=f32)[None, :], (128, 1)).copy()

    maps = []
    for core in range(cfg.NC):
        m = {"attT": attT, "basisf": basisf, "fcrT": fcrT, "fcb2": fcb2, "ic": IC}
        s0 = host_prep_streams(cfg, drug_feat, cj_drug, ci_dis, src, dst, core)
        s1 = host_prep_streams(cfg, dis_feat, cj_dis, ci_drug, dst, src, core)
        for d, s in ((0, s0), (1, s1)):
            m[f"d{d}sl"], m[f"d{d}sc"] = s["sl"], s["sc"]
            for k in range(NK):
                m[f"d{d}g{k}"] = s[f"g{k}"]
        maps.append(m)
    return maps


def assemble_output(cfg, results):
    dis_out = np.concatenate(
        [results[c]["outT"][0].T.astype(np.float32) for c in range(cfg.NC)], 0)
    drug_out = np.concatenate(
        [results[c]["outT"][1].T.astype(np.float32) for c in range(cfg.NC)], 0)
    return drug_out, dis_out


def build_kernel(cfg, debug=True):
    nc = bacc.Bacc(None, target_bir_lowering=False, debug=debug)
    IN, NCALL, NWP = cfg.IN, cfg.NCALL, cfg.NWP
    NPAGE, WPP, SH, PPR = cfg.NPAGE, cfg.WPP, cfg.SH, cfg.PPR
    NSLOT, DCH, NDC, SPS, NSTG = cfg.NSLOT, cfg.DCH, cfg.NDC, cfg.SPS, cfg.NSTG
    WROUND = cfg.WROUND
    PASSES = [(d, k) for d in range(2) for k in range(NK)]

    attT_d = nc.declare_dram_parameter("attT", [4, R], F32, isOutput=False)
    basisf_d = nc.declare_dram_parameter("basisf", [4, IN * MU], F32, isOutput=False)
    fcr_d = nc.declare_dram_parameter("fcrT", [MU, R * NK, OUT], BF16, isOutput=False)
    fcb_d = nc.declare_dram_parameter("fcb2", [128, 2], F32, isOutput=False)
    ic_d = nc.declare_dram_parameter("ic", [128, 128], F32, isOutput=False)
    gD, slD, scD = {}, {}, {}
    for d, k in PASSES:
        gD[d, k] = nc.declare_dram_parameter(f"d{d}g{k}", [16, NCALL * 64], I16, isOutput=False)
    for d in range(2):
        slD[d] = nc.declare_dram_parameter(f"d{d}sl", [128, NWP], U8, isOutput=False)
        scD[d] = nc.declare_dram_parameter(f"d{d}sc", [128, NWP], BF16, isOutput=False)
    outT_d = nc.declare_dram_parameter("outT", [2, OUT, SH], F16, isOutput=True)

    wtab = nc.dram_tensor("wtab", [R, IN * MU], F32)
    wtab_rows = wtab[:].rearrange("r (f m) -> (r f) m", m=MU)
    hT = nc.dram_tensor("hT", [2, NK, MU, NSLOT], BF16)

    attT_sb = nc.alloc_sbuf_tensor("attT_sb", [4, R], F32)
    bchunk = nc.alloc_sbuf_tensor("bchunk", [4, 2048], F32)
    wstage = nc.alloc_sbuf_tensor("wstage", [R, 2048], F32)
    ic_sb = nc.alloc_sbuf_tensor("ic_sb", [128, 128], F32)
    fcr_sb = nc.alloc_sbuf_tensor("fcr_sb", [MU, R * NK, OUT], BF16)
    fcb_sb = nc.alloc_sbuf_tensor("fcb_sb", [128, 2], F32)
    gsb = nc.alloc_sbuf_tensor("gsb", [128, NCALL * 64], I16)
    slr = nc.alloc_sbuf_tensor("slr", [128, NWP], U8)
    scr = nc.alloc_sbuf_tensor("scr", [128, NWP], BF16)
    slf = nc.alloc_sbuf_tensor("slf", [128, NWP], F32)
    scf = nc.alloc_sbuf_tensor("scf", [128, NWP], F32)
    NMB = 4
    msgs = [nc.alloc_sbuf_tensor(f"msgs{i}", [128, 8, MU], F32) for i in range(NMB)]
    # segt is built page-at-a-time: WPP windows, plus the tail page which also
    # absorbs the NWP-NPAGE*WPP pad windows.
    WLAST = WPP + (cfg.NWP - cfg.NPAGE * cfg.WPP)
    NSB = 2
    segt = [nc.alloc_sbuf_tensor(f"segt{i}", [128, WLAST, 128], F32)
            for i in range(NSB)]
    NSTB = 2
    stage = [nc.alloc_sbuf_tensor(f"stage{i}", [MU, SPS * 128], BF16) for i in range(NSTB)]
    prhs = [nc.alloc_sbuf_tensor(f"prhs{i}", [MU, R * NK, DCH], BF16) for i in range(2)]
    ostage = [nc.alloc_sbuf_tensor(f"ostage{i}", [128, DCH], F16) for i in range(4)]

    NPB = 4
    psA = nc.alloc_psum_tensor("psA", [128, 2048], F32)
    pages = [psA[0:MU, i * 512:i * 512 + 128] for i in range(NPB)]
    psB = nc.alloc_psum_tensor("psB", [128, 2048], F32)
    wps = psB[0:R, :]
    pps = [psB[:, j * 512:(j + 1) * 512] for j in range(4)]  # j = 2*(l%2)+h

    def page_of_window(w):
        return min(w // WPP, NPAGE - 1)

    wlast = {}
    for pi in range(len(PASSES)):
        for w in range(NWP):
            wlast[pi, page_of_window(w)] = pi * NWP + w

    sems = {}

    with nc.Block() as block:
        for name, n in [("gth", NMB), ("stg", NSTB), ("pin", 2), ("ost", 4)]:
            for i in range(n):
                sems[name, i] = nc.alloc_semaphore(f"s_{name}{i}")
        for name in ["wdma", "wout", "wmm", "wcp", "seg", "pe", "act", "pmm",
                     "oact", "gsb", "slraw"]:
            sems[name] = nc.alloc_semaphore(f"s_{name}")
        s_gth = [sems["gth", i] for i in range(NMB)]
        s_stg = [sems["stg", i] for i in range(NSTB)]
        s_pin = [sems["pin", i] for i in range(2)]
        s_ost = [sems["ost", i] for i in range(4)]
        s_wdma, s_wmm, s_wcp = sems["wdma"], sems["wmm"], sems["wcp"]
        s_wout = sems["wout"]
        s_seg, s_pe, s_act = sems["seg"], sems["pe"], sems["act"]
        s_pmm, s_oact = sems["pmm"], sems["oact"]
        s_gsb, s_slraw = sems["gsb"], sems["slraw"]

        # ============ GPSIMD: const + W-build DMAs, then gathers
        @block.gpsimd
        def _(g):
            g.dma_start(attT_sb[:], attT_d[:]).then_inc(s_wdma, 16)
            g.dma_start(ic_sb[:], ic_d[:]).then_inc(s_wdma, 16)
            g.dma_start(fcr_sb[:], fcr_d[:]).then_inc(s_wdma, 16)
            g.dma_start(fcb_sb[:], fcb_d[:]).then_inc(s_wdma, 16)
            for n in range(WROUND):
                g.wait_ge(s_wcp, n)  # round n-1 psum copied (wstage free after out-DMA below)
                g.dma_start(bchunk[:], basisf_d[:, n*2048:(n+1)*2048]).then_inc(s_wdma, 16)
                g.wait_ge(s_wcp, n + 1)
                g.dma_start(wtab[:, n*2048:(n+1)*2048], wstage[:]).then_inc(s_wout, 16)
            g.wait_ge(s_wout, WROUND * 16)  # all wtab writes landed
            ncall = 0
            for pi, (d, k) in enumerate(PASSES):
                g.wait_ge(s_gsb, 128 * (pi + 1))  # this pass's gsb loaded
                for c in range(NCALL):
                    b = ncall % NMB
                    if ncall >= NMB:
                        g.wait_ge(s_pe, 8 * (ncall - NMB + 1))
                    g.dma_gather(
                        msgs[b][:], wtab_rows,
                        gsb[:, c * 64:(c + 1) * 64],
                        1024, 1024, MU,
                    ).then_inc(s_gth[b], 16)
                    ncall += 1

        # ============ TENSOR: W MMs, window MMs, projection MMs
        @block.tensor
        def _(t):
            for n in range(WROUND):
                t.wait_ge(s_wdma, 64 + 16 * (n + 1))
                if n >= 1:
                    t.wait_ge(s_wcp, n)
                for i in range(4):
                    ins = t.matmul(wps[:, i*512:(i+1)*512], attT_sb[:],
                                   bchunk[:, i*512:(i+1)*512],
                                   start=True, stop=True)
                ins.then_inc(s_wmm, 1)
            wglob = 0
            for pi, (d, k) in enumerate(PASSES):
                for w in range(NWP):
                    p = page_of_window(w)
                    pglob = pi * NPAGE + p
                    first = (w % WPP == 0) and (p == w // WPP)
                    if first and pglob >= NPB:
                        t.wait_ge(s_act, pglob - NPB + 1)
                    b = (wglob // 8) % NMB
                    t.wait_ge(s_gth[b], 16 * (wglob // 8 // NMB + 1))
                    if first or w - p * WPP == 0:
                        t.wait_ge(s_seg, pglob + 1)
                    t.matmul(pages[pglob % NPB],
                             msgs[b][:, w % 8, :],
                             segt[pglob % NSB][:, w - p * WPP, :],
                             start=first, stop=(wglob == wlast[pi, p]),
                             ).then_inc(s_pe, 1)
                    wglob += 1
            nl = 0
            for d in range(2):
                for c in range(NDC):
                    ncols = min(DCH, SH - c * DCH)
                    t.wait_ge(s_pin[nl % 2], 240 * (nl // 2 + 1))
                    if nl >= 2:
                        t.wait_ge(s_oact, 2 * (nl - 1))
                    for h in range(2):
                        for rk in range(R * NK):
                            ins = t.matmul(pps[2*(nl % 2)+h][:, :ncols],
                                           fcr_sb[:, rk, h*128:(h+1)*128],
                                           prhs[nl % 2][:, rk, :ncols],
                                           start=(rk == 0), stop=(rk == R*NK-1))
                        ins.then_inc(s_pmm, 1)
                    nl += 1

        # ============ VECTOR: W psum->sbuf copies, sl/sc converts, SegT builds
        @block.vector
        def _(v):
            for n in range(WROUND):
                v.wait_ge(s_wmm, n + 1)
                if n >= 1:
                    v.wait_ge(s_wout, 16 * n)
                v.tensor_copy(wstage[:], wps[:]).then_inc(s_wcp, 1)
            for pi, (d, k) in enumerate(PASSES):
                if k == 0:
                    # direction start: widen sl u8 / sc bf16 to f32 once
                    v.wait_ge(s_slraw, 32 * (d + 1))
                    v.tensor_copy(slf[:], slr[:])
                    v.tensor_copy(scf[:], scr[:])
                for p in range(NPAGE):
                    nwin = WPP if p < NPAGE - 1 else WLAST
                    pglob = pi * NPAGE + p
                    if pglob >= NSB:
                        pprev = pglob - NSB
                        v.wait_ge(s_pe, wlast[pprev // NPAGE, pprev % NPAGE] + 1)
                    sb = segt[pglob % NSB][:, 0:nwin, :]
                    ic_b = ic_sb[:].unsqueeze(1).to_broadcast([128, nwin, 128])
                    sl_b = slf[:, p*WPP:p*WPP+nwin].unsqueeze(2).to_broadcast(
                        [128, nwin, 128])
                    sc_b = scf[:, p*WPP:p*WPP+nwin].unsqueeze(2).to_broadcast(
                        [128, nwin, 128])
                    v.scalar_tensor_tensor(
                        sb, ic_b, 0.0, sl_b,
                        mybir.AluOpType.bypass, mybir.AluOpType.is_equal)
                    v.scalar_tensor_tensor(
                        sb, sb, 0.0, sc_b,
                        mybir.AluOpType.bypass, mybir.AluOpType.mult,
                    ).then_inc(s_seg, 1)
            nl = 0
            for d in range(2):
                for c in range(NDC):
                    ncols = min(DCH, SH - c * DCH)
                    for h in range(2):
                        ob = 2 * (nl % 2) + h
                        v.wait_ge(s_pmm, 2 * nl + h + 1)
                        if nl >= 2:
                            v.wait_ge(s_ost[ob], 16 * (nl // 2))
                        v.tensor_scalar(
                            ostage[ob][:, :ncols], pps[ob][:, :ncols],
                            fcb_sb[:, h:h+1], None,
                            mybir.AluOpType.add,
                        ).then_inc(s_oact, 1)
                    nl += 1

        # ============ SCALAR: page->stage copies (bf16); stage->hT DMAs
        @block.scalar
        def _(a):
            pglob = 0
            for pi, (d, k) in enumerate(PASSES):
                for p in range(NPAGE):
                    st = p // SPS
                    stglob = pi * NSTG + st
                    a.wait_ge(s_pe, wlast[pi, p] + 1)
                    if stglob >= NSTB and p % SPS == 0:
                        a.wait_ge(s_stg[stglob % NSTB], 16 * (stglob // NSTB))
                    a.copy(stage[stglob % NSTB][:, (p % SPS)*128:(p % SPS+1)*128],
                           pages[pglob % NPB]).then_inc(s_act, 1)
                    pglob += 1
                    if p % SPS == SPS - 1 or p == NPAGE - 1:
                        p0 = st * SPS
                        npg = p - p0 + 1
                        a.wait_ge(s_act, pglob)
                        a.dma_start(hT[d, k][:, p0*128:(p0+npg)*128],
                                    stage[stglob % NSTB][:, :npg*128]
                                    ).then_inc(s_stg[stglob % NSTB], 16)

        # ============ SYNC: g replication + sl/sc loads, proj loads, out DMAs
        @block.sync
        def _(s):
            for pi, (d, k) in enumerate(PASSES):
                if pi >= 1:
                    s.wait_ge(s_pe, pi * NWP)   # gsb (and for k==0, slr/scr) free
                if k == 0:
                    s.dma_start(slr[:], slD[d][:]).then_inc(s_slraw, 16)
                    s.dma_start(scr[:], scD[d][:]).then_inc(s_slraw, 16)
                for rep in range(8):
                    s.dma_start(gsb[rep*16:(rep+1)*16, :], gD[d, k][:]
                                ).then_inc(s_gsb, 16)
            # wait all stage->hT DMAs before projection loads
            NSTGALL = len(PASSES) * NSTG
            for b in range(NSTB):
                occ = (NSTGALL - b + NSTB - 1) // NSTB
                s.wait_ge(s_stg[b], 16 * occ)
            nl = 0
            for d in range(2):
                for c in range(NDC):
                    ncols = min(DCH, SH - c * DCH)
                    if nl >= 2:
                        s.wait_ge(s_pmm, 2 * (nl - 1))
                    for rk in range(R * NK):
                        r, k = rk // NK, rk % NK
                        s.dma_start(
                            prhs[nl % 2][:, rk, :ncols],
                            hT[d, k][:, r*PPR*128 + c*DCH: r*PPR*128 + c*DCH + ncols]
                        ).then_inc(s_pin[nl % 2], 16)
                    for h in range(2):
                        ob = 2 * (nl % 2) + h
                        s.wait_ge(s_oact, 2 * nl + h + 1)
                        s.dma_start(outT_d[d, h*128:(h+1)*128, c*DCH:c*DCH+ncols],
                                    ostage[ob][:, :ncols]).then_inc(s_ost[ob], 16)
                    nl += 1
            NLD = 2 * NDC
            for b in range(2):
                occ = (NLD - b + 1) // 2
                for h in range(2):
                    s.wait_ge(s_ost[2 * b + h], 16 * occ)

    nc.compile()
    return nc


# ======================================================================
# Self-contained kernel entry point.
# ======================================================================
from concourse.bass_utils import run_bass_kernel_spmd as _run_spmd

_CACHE = {}


def kernel(**inputs):
    """GCMC layer on 8 trn2 NeuronCores. Returns (drug_out, dis_out) f32."""
    cfg = Cfg(50000, 1024, 8, wpp=12)
    maps = build_inputs(cfg, inputs)
    if "nc" not in _CACHE:
        _CACHE["nc"] = build_kernel(cfg)
    res = _run_spmd(_CACHE["nc"], maps, list(range(cfg.NC)))
    return assemble_output(cfg, res.results)


# revision 27
# speedup vs baseline: 5.2450x; 1.1764x over previous
"""GCMC message-passing kernel for trn2: builder + host preprocessing.

Per core = one dst-shard, both directions (0: drug->dis, 1: dis->drug).
  Phase W: device computes W[r] = att @ basis -> wtab[R, IN*MU] f32 in HBM.
  Phase E (x6 passes = 2 dirs x 3 k-feats): per-edge event streams sorted by
    slot (r-major, dst-local), 128-event windows, WPP windows per 128-slot
    page. dma_gather pulls 64-f32 W rows (1024 events/call); DVE builds
    SegT[128ev,128slot] = is_equal(IC, sl) * sc  (sc = cj*ci, host-folded);
    PE: msgs.T @ SegT accumulated into a PSUM page [MU, 128].
    Pages -> SBUF stage (ACT, bf16) -> hT[d,k] = [MU, NSLOT] bf16 HBM (SYNC).
  Phase P: outT[d] [256, SH] = sum_rk fcblk_rk.T @ hT-slices + bias (f16 out).

Wire-format choices (the axon tunnel is ~50MB/s, so bytes dominate wall):
  gD[d,k]  int16 [16, NCALL*64]  -- un-replicated; device copies it into all
                                    eight 16-partition groups of the SBUF
                                    index buffer (dma_gather wants 8 replicas).
  slD[d]   uint8 [128, NWP]      -- slot-in-page, shared across the 3 k-passes.
  scD[d]   bf16  [128, NWP]      -- cj*ci edge scale, shared across k-passes.
  fcrT     bf16, hT staging bf16, outT float16.
Host assembles + transposes the two outputs (float32).
"""
import numpy as np
import ml_dtypes
import concourse.bass as bass
import concourse.bacc as bacc
import concourse.mybir as mybir

F32 = mybir.dt.float32
F16 = mybir.dt.float16
BF16 = mybir.dt.bfloat16
I16 = mybir.dt.int16
U8 = mybir.dt.uint8

NP_BF16 = ml_dtypes.bfloat16

R = 5
MU = 64
OUT = 256
NK = 3


class Cfg:
    def __init__(self, n_nodes, in_units, n_cores, wpp):
        self.N = n_nodes
        self.IN = in_units
        self.NC = n_cores
        self.SH = n_nodes // n_cores
        self.PPR = (self.SH + 127) // 128
        self.NPAGE = R * self.PPR
        self.NSLOT = self.NPAGE * 128
        self.WPP = wpp
        self.NW = self.NPAGE * wpp
        self.NWP = ((self.NW + 7) // 8) * 8
        self.NCALL = self.NWP // 8
        self.DCH = 512
        self.NDC = (self.SH + self.DCH - 1) // self.DCH
        self.SPS = 16
        self.NSTG = (self.NPAGE + self.SPS - 1) // self.SPS
        self.WROUND = (in_units * MU) // 2048
        assert (in_units * MU) % 2048 == 0


def host_prep_streams(cfg, feat_src, cj_src, ci_dst, src, dst, core):
    SH, PPR, WPP = cfg.SH, cfg.PPR, cfg.WPP
    lo, hi = core * SH, (core + 1) * SH
    evg = [[] for _ in range(NK)]
    evsl, evsc = [], []
    for r in range(R):
        m = (dst[r] >= lo) & (dst[r] < hi)
        s, d = src[r][m], dst[r][m]
        slot = r * PPR * 128 + (d - lo)
        sc = (cj_src[s, 0] * ci_dst[d, 0]).astype(np.float32)
        for k in range(NK):
            evg[k].append((r * cfg.IN + feat_src[s, k]).astype(np.int64))
        evsl.append(slot.astype(np.int64))
        evsc.append(sc)
    sl = np.concatenate(evsl)
    sc = np.concatenate(evsc)
    order = np.argsort(sl, kind="stable")
    sl, sc = sl[order], sc[order]
    NWP = cfg.NWP
    SL = np.zeros((NWP, 128), np.uint8)
    SC = np.zeros((NWP, 128), np.float32)
    page = sl // 128
    counts = np.bincount(page, minlength=cfg.NPAGE)
    assert counts.max() <= WPP * 128, (
        f"page overflow {counts.max()} > {WPP*128}; raise WPP")
    # window boundaries, shared by all three k streams
    bounds = []
    pos = 0
    for p in range(cfg.NPAGE):
        n = counts[p]
        sls, scs = sl[pos:pos+n], sc[pos:pos+n]
        for w in range((n + 127) // 128):
            a, b = w * 128, min((w + 1) * 128, n)
            wi = p * WPP + w
            SL[wi, :b-a] = (sls[a:b] - p * 128).astype(np.uint8)
            SC[wi, :b-a] = scs[a:b]
            bounds.append((wi, pos + a, pos + b))
        pos += n
    out = {
        "sl": SL.T.copy(),
        "sc": SC.T.astype(NP_BF16).copy(),
    }
    for k in range(NK):
        g = np.concatenate(evg[k])[order]
        G = np.zeros((NWP, 128), np.int16)
        for wi, a, b in bounds:
            G[wi, :b-a] = g[a:b]
        Gc = G.reshape(cfg.NCALL, 1024)
        wr = np.zeros((16, cfg.NCALL * 64), np.int16)
        for c in range(cfg.NCALL):
            wr[:, c*64:(c+1)*64] = Gc[c].reshape(64, 16).T
        out[f"g{k}"] = wr
    return out


def build_inputs(cfg, inputs):
    f32 = np.float32
    gi = lambda n: np.asarray(inputs[n], np.int64)
    gf = lambda n: np.asarray(inputs[n], f32)
    drug_feat, dis_feat = gi("drug_feat"), gi("dis_feat")
    src, dst = gi("src"), gi("dst")
    cj_drug, ci_drug = gf("cj_drug"), gf("ci_drug")
    cj_dis, ci_dis = gf("cj_dis"), gf("ci_dis")
    att, basis = gf("att"), gf("basis")
    fc_w, fc_b = gf("fc_w"), gf("fc_b")

    attT = att.T.copy()
    basisf = basis.reshape(4, cfg.IN * MU).copy()
    # fcrT[m, rk, o] = fc_w[r*NK*MU + k*MU + m, o]
    fcrT = fc_w.reshape(R * NK, MU, OUT).transpose(1, 0, 2).astype(NP_BF16).copy()
    fcb2 = fc_b.reshape(2, 128).T.copy()      # [128, 2], col h = half h
    IC = np.tile(np.arange(128, dtype=f32)[None, :], (128, 1)).copy()

    maps = []
    for core in range(cfg.NC):
        m = {"attT": attT, "basisf": basisf, "fcrT": fcrT, "fcb2": fcb2, "ic": IC}
        s0 = host_prep_streams(cfg, drug_feat, cj_drug, ci_dis, src, dst, core)
        s1 = host_prep_streams(cfg, dis_feat, cj_dis, ci_drug, dst, src, core)
        for d, s in ((0, s0), (1, s1)):
            m[f"d{d}sl"], m[f"d{d}sc"] = s["sl"], s["sc"]
            for k in range(NK):
                m[f"d{d}g{k}"] = s[f"g{k}"]
        maps.append(m)
    return maps


def assemble_output(cfg, results):
    dis_out = np.concatenate(
        [results[c]["outT"][0].T.astype(np.float32) for c in range(cfg.NC)], 0)
    drug_out = np.concatenate(
        [results[c]["outT"][1].T.astype(np.float32) for c in range(cfg.NC)], 0)
    return drug_out, dis_out


def build_kernel(cfg, debug=True):
    nc = bacc.Bacc(None, target_bir_lowering=False, debug=debug)
    IN, NCALL, NWP = cfg.IN, cfg.NCALL, cfg.NWP
    NPAGE, WPP, SH, PPR = cfg.NPAGE, cfg.WPP, cfg.SH, cfg.PPR
    NSLOT, DCH, NDC, SPS, NSTG = cfg.NSLOT, cfg.DCH, cfg.NDC, cfg.SPS, cfg.NSTG
    WROUND = cfg.WROUND
    PASSES = [(d, k) for d in range(2) for k in range(NK)]

    attT_d = nc.declare_dram_parameter("attT", [4, R], F32, isOutput=False)
    basisf_d = nc.declare_dram_parameter("basisf", [4, IN * MU], F32, isOutput=False)
    fcr_d = nc.declare_dram_parameter("fcrT", [MU, R * NK, OUT], BF16, isOutput=False)
    fcb_d = nc.declare_dram_parameter("fcb2", [128, 2], F32, isOutput=False)
    ic_d = nc.declare_dram_parameter("ic", [128, 128], F32, isOutput=False)
    gD, slD, scD = {}, {}, {}
    for d, k in PASSES:
        gD[d, k] = nc.declare_dram_parameter(f"d{d}g{k}", [16, NCALL * 64], I16, isOutput=False)
    for d in range(2):
        slD[d] = nc.declare_dram_parameter(f"d{d}sl", [128, NWP], U8, isOutput=False)
        scD[d] = nc.declare_dram_parameter(f"d{d}sc", [128, NWP], BF16, isOutput=False)
    outT_d = nc.declare_dram_parameter("outT", [2, OUT, SH], F16, isOutput=True)

    wtab = nc.dram_tensor("wtab", [R, IN * MU], F32)
    wtab_rows = wtab[:].rearrange("r (f m) -> (r f) m", m=MU)
    hT = nc.dram_tensor("hT", [2, NK, MU, NSLOT], BF16)

    attT_sb = nc.alloc_sbuf_tensor("attT_sb", [4, R], F32)
    bchunk = nc.alloc_sbuf_tensor("bchunk", [4, 2048], F32)
    wstage = nc.alloc_sbuf_tensor("wstage", [R, 2048], F32)
    ic_sb = nc.alloc_sbuf_tensor("ic_sb", [128, 128], F32)
    fcr_sb = nc.alloc_sbuf_tensor("fcr_sb", [MU, R * NK, OUT], BF16)
    fcb_sb = nc.alloc_sbuf_tensor("fcb_sb", [128, 2], F32)
    gsb = nc.alloc_sbuf_tensor("gsb", [128, NCALL * 64], I16)
    slr = nc.alloc_sbuf_tensor("slr", [128, NWP], U8)
    scr = nc.alloc_sbuf_tensor("scr", [128, NWP], BF16)
    slf = nc.alloc_sbuf_tensor("slf", [128, NWP], F32)
    scf = nc.alloc_sbuf_tensor("scf", [128, NWP], F32)
    NMB = 8
    msgs = [nc.alloc_sbuf_tensor(f"msgs{i}", [128, 8, MU], F32) for i in range(NMB)]
    # segt is built page-at-a-time: WPP windows, plus the tail page which also
    # absorbs the NWP-NPAGE*WPP pad windows.
    WLAST = WPP + (cfg.NWP - cfg.NPAGE * cfg.WPP)
    NSB = 3
    segt = [nc.alloc_sbuf_tensor(f"segt{i}", [128, WLAST, 128], F32)
            for i in range(NSB)]
    NSTB = 2
    stage = [nc.alloc_sbuf_tensor(f"stage{i}", [MU, SPS * 128], BF16) for i in range(NSTB)]
    prhs = [nc.alloc_sbuf_tensor(f"prhs{i}", [MU, R * NK, DCH], BF16) for i in range(2)]
    ostage = [nc.alloc_sbuf_tensor(f"ostage{i}", [128, DCH], F16) for i in range(4)]

    NPB = 4
    psA = nc.alloc_psum_tensor("psA", [128, 2048], F32)
    pages = [psA[0:MU, i * 512:i * 512 + 128] for i in range(NPB)]
    psB = nc.alloc_psum_tensor("psB", [128, 2048], F32)
    wps = psB[0:R, :]
    pps = [psB[:, j * 512:(j + 1) * 512] for j in range(4)]  # j = 2*(l%2)+h

    def page_of_window(w):
        return min(w // WPP, NPAGE - 1)

    wlast = {}
    for pi in range(len(PASSES)):
        for w in range(NWP):
            wlast[pi, page_of_window(w)] = pi * NWP + w

    sems = {}

    with nc.Block() as block:
        for name, n in [("gth", NMB), ("stg", NSTB), ("pin", 2), ("ost", 4)]:
            for i in range(n):
                sems[name, i] = nc.alloc_semaphore(f"s_{name}{i}")
        for name in ["wdma", "wout", "wmm", "wcp", "seg", "pe", "act", "pmm",
                     "oact", "gsb", "slraw"]:
            sems[name] = nc.alloc_semaphore(f"s_{name}")
        s_gth = [sems["gth", i] for i in range(NMB)]
        s_stg = [sems["stg", i] for i in range(NSTB)]
        s_pin = [sems["pin", i] for i in range(2)]
        s_ost = [sems["ost", i] for i in range(4)]
        s_wdma, s_wmm, s_wcp = sems["wdma"], sems["wmm"], sems["wcp"]
        s_wout = sems["wout"]
        s_seg, s_pe, s_act = sems["seg"], sems["pe"], sems["act"]
        s_pmm, s_oact = sems["pmm"], sems["oact"]
        s_gsb, s_slraw = sems["gsb"], sems["slraw"]

        # ============ GPSIMD: const + W-build DMAs, then gathers
        @block.gpsimd
        def _(g):
            g.dma_start(attT_sb[:], attT_d[:]).then_inc(s_wdma, 16)
            g.dma_start(ic_sb[:], ic_d[:]).then_inc(s_wdma, 16)
            g.dma_start(fcr_sb[:], fcr_d[:]).then_inc(s_wdma, 16)
            g.dma_start(fcb_sb[:], fcb_d[:]).then_inc(s_wdma, 16)
            for n in range(WROUND):
                g.wait_ge(s_wcp, n)  # round n-1 psum copied (wstage free after out-DMA below)
                g.dma_start(bchunk[:], basisf_d[:, n*2048:(n+1)*2048]).then_inc(s_wdma, 16)
                g.wait_ge(s_wcp, n + 1)
                g.dma_start(wtab[:, n*2048:(n+1)*2048], wstage[:]).then_inc(s_wout, 16)
            g.wait_ge(s_wout, WROUND * 16)  # all wtab writes landed
            ncall = 0
            for pi, (d, k) in enumerate(PASSES):
                g.wait_ge(s_gsb, 128 * (pi + 1))  # this pass's gsb loaded
                for c in range(NCALL):
                    b = ncall % NMB
                    if ncall >= NMB:
                        g.wait_ge(s_pe, 8 * (ncall - NMB + 1))
                    g.dma_gather(
                        msgs[b][:], wtab_rows,
                        gsb[:, c * 64:(c + 1) * 64],
                        1024, 1024, MU,
                    ).then_inc(s_gth[b], 16)
                    ncall += 1

        # ============ TENSOR: W MMs, window MMs, projection MMs
        @block.tensor
        def _(t):
            for n in range(WROUND):
                t.wait_ge(s_wdma, 64 + 16 * (n + 1))
                if n >= 1:
                    t.wait_ge(s_wcp, n)
                for i in range(4):
                    ins = t.matmul(wps[:, i*512:(i+1)*512], attT_sb[:],
                                   bchunk[:, i*512:(i+1)*512],
                                   start=True, stop=True)
                ins.then_inc(s_wmm, 1)
            wglob = 0
            for pi, (d, k) in enumerate(PASSES):
                for w in range(NWP):
                    p = page_of_window(w)
                    pglob = pi * NPAGE + p
                    first = (w % WPP == 0) and (p == w // WPP)
                    if first and pglob >= NPB:
                        t.wait_ge(s_act, pglob - NPB + 1)
                    b = (wglob // 8) % NMB
                    t.wait_ge(s_gth[b], 16 * (wglob // 8 // NMB + 1))
                    if first or w - p * WPP == 0:
                        t.wait_ge(s_seg, pglob + 1)
                    t.matmul(pages[pglob % NPB],
                             msgs[b][:, w % 8, :],
                             segt[pglob % NSB][:, w - p * WPP, :],
                             start=first, stop=(wglob == wlast[pi, p]),
                             ).then_inc(s_pe, 1)
                    wglob += 1
            nl = 0
            for d in range(2):
                for c in range(NDC):
                    ncols = min(DCH, SH - c * DCH)
                    t.wait_ge(s_pin[nl % 2], 240 * (nl // 2 + 1))
                    if nl >= 2:
                        t.wait_ge(s_oact, 2 * (nl - 1))
                    for h in range(2):
                        for rk in range(R * NK):
                            ins = t.matmul(pps[2*(nl % 2)+h][:, :ncols],
                                           fcr_sb[:, rk, h*128:(h+1)*128],
                                           prhs[nl % 2][:, rk, :ncols],
                                           start=(rk == 0), stop=(rk == R*NK-1))
                        ins.then_inc(s_pmm, 1)
                    nl += 1

        # ============ VECTOR: W psum->sbuf copies, sl/sc converts, SegT builds
        @block.vector
        def _(v):
            for n in range(WROUND):
                v.wait_ge(s_wmm, n + 1)
                if n >= 1:
                    v.wait_ge(s_wout, 16 * n)
                v.tensor_copy(wstage[:], wps[:]).then_inc(s_wcp, 1)
            for pi, (d, k) in enumerate(PASSES):
                if k == 0:
                    # direction start: widen sl u8 / sc bf16 to f32 once
                    v.wait_ge(s_slraw, 32 * (d + 1))
                    v.tensor_copy(slf[:], slr[:])
                    v.tensor_copy(scf[:], scr[:])
                for p in range(NPAGE):
                    nwin = WPP if p < NPAGE - 1 else WLAST
                    pglob = pi * NPAGE + p
                    if pglob >= NSB:
                        pprev = pglob - NSB
                        v.wait_ge(s_pe, wlast[pprev // NPAGE, pprev % NPAGE] + 1)
                    sb = segt[pglob % NSB][:, 0:nwin, :]
                    ic_b = ic_sb[:].unsqueeze(1).to_broadcast([128, nwin, 128])
                    sl_b = slf[:, p*WPP:p*WPP+nwin].unsqueeze(2).to_broadcast(
                        [128, nwin, 128])
                    sc_b = scf[:, p*WPP:p*WPP+nwin].unsqueeze(2).to_broadcast(
                        [128, nwin, 128])
                    v.scalar_tensor_tensor(
                        sb, ic_b, 0.0, sl_b,
                        mybir.AluOpType.bypass, mybir.AluOpType.is_equal)
                    v.scalar_tensor_tensor(
                        sb, sb, 0.0, sc_b,
                        mybir.AluOpType.bypass, mybir.AluOpType.mult,
                    ).then_inc(s_seg, 1)
            nl = 0
            for d in range(2):
                for c in range(NDC):
                    ncols = min(DCH, SH - c * DCH)
                    for h in range(2):
                        ob = 2 * (nl % 2) + h
                        v.wait_ge(s_pmm, 2 * nl + h + 1)
                        if nl >= 2:
                            v.wait_ge(s_ost[ob], 16 * (nl // 2))
                        v.tensor_scalar(
                            ostage[ob][:, :ncols], pps[ob][:, :ncols],
                            fcb_sb[:, h:h+1], None,
                            mybir.AluOpType.add,
                        ).then_inc(s_oact, 1)
                    nl += 1

        # ============ SCALAR: page->stage copies (bf16); stage->hT DMAs
        @block.scalar
        def _(a):
            pglob = 0
            for pi, (d, k) in enumerate(PASSES):
                for p in range(NPAGE):
                    st = p // SPS
                    stglob = pi * NSTG + st
                    a.wait_ge(s_pe, wlast[pi, p] + 1)
                    if stglob >= NSTB and p % SPS == 0:
                        a.wait_ge(s_stg[stglob % NSTB], 16 * (stglob // NSTB))
                    a.copy(stage[stglob % NSTB][:, (p % SPS)*128:(p % SPS+1)*128],
                           pages[pglob % NPB]).then_inc(s_act, 1)
                    pglob += 1
                    if p % SPS == SPS - 1 or p == NPAGE - 1:
                        p0 = st * SPS
                        npg = p - p0 + 1
                        a.wait_ge(s_act, pglob)
                        a.dma_start(hT[d, k][:, p0*128:(p0+npg)*128],
                                    stage[stglob % NSTB][:, :npg*128]
                                    ).then_inc(s_stg[stglob % NSTB], 16)

        # ============ SYNC: g replication + sl/sc loads, proj loads, out DMAs
        @block.sync
        def _(s):
            for pi, (d, k) in enumerate(PASSES):
                if pi >= 1:
                    s.wait_ge(s_pe, pi * NWP)   # gsb (and for k==0, slr/scr) free
                if k == 0:
                    s.dma_start(slr[:], slD[d][:]).then_inc(s_slraw, 16)
                    s.dma_start(scr[:], scD[d][:]).then_inc(s_slraw, 16)
                for rep in range(8):
                    s.dma_start(gsb[rep*16:(rep+1)*16, :], gD[d, k][:]
                                ).then_inc(s_gsb, 16)
            # wait all stage->hT DMAs before projection loads
            NSTGALL = len(PASSES) * NSTG
            for b in range(NSTB):
                occ = (NSTGALL - b + NSTB - 1) // NSTB
                s.wait_ge(s_stg[b], 16 * occ)
            nl = 0
            for d in range(2):
                for c in range(NDC):
                    ncols = min(DCH, SH - c * DCH)
                    if nl >= 2:
                        s.wait_ge(s_pmm, 2 * (nl - 1))
                    for rk in range(R * NK):
                        r, k = rk // NK, rk % NK
                        s.dma_start(
                            prhs[nl % 2][:, rk, :ncols],
                            hT[d, k][:, r*PPR*128 + c*DCH: r*PPR*128 + c*DCH + ncols]
                        ).then_inc(s_pin[nl % 2], 16)
                    for h in range(2):
                        ob = 2 * (nl % 2) + h
                        s.wait_ge(s_oact, 2 * nl + h + 1)
                        s.dma_start(outT_d[d, h*128:(h+1)*128, c*DCH:c*DCH+ncols],
                                    ostage[ob][:, :ncols]).then_inc(s_ost[ob], 16)
                    nl += 1
            NLD = 2 * NDC
            for b in range(2):
                occ = (NLD - b + 1) // 2
                for h in range(2):
                    s.wait_ge(s_ost[2 * b + h], 16 * occ)

    nc.compile()
    return nc


# ======================================================================
# Self-contained kernel entry point.
# ======================================================================
from concourse.bass_utils import run_bass_kernel_spmd as _run_spmd

_CACHE = {}


def kernel(**inputs):
    """GCMC layer on 8 trn2 NeuronCores. Returns (drug_out, dis_out) f32."""
    cfg = Cfg(50000, 1024, 8, wpp=12)
    maps = build_inputs(cfg, inputs)
    if "nc" not in _CACHE:
        _CACHE["nc"] = build_kernel(cfg)
    res = _run_spmd(_CACHE["nc"], maps, list(range(cfg.NC)))
    return assemble_output(cfg, res.results)


# revision 35
# speedup vs baseline: 5.2479x; 1.0006x over previous
"""GCMC message-passing kernel for trn2: builder + host preprocessing.

Per core = one dst-shard, both directions (0: drug->dis, 1: dis->drug).
  Phase W: device computes W[r] = att @ basis -> wtab[R, IN*MU] f32 in HBM.
  Phase E (x6 passes = 2 dirs x 3 k-feats): per-edge event streams sorted by
    slot (r-major, dst-local), 128-event windows, WPP windows per 128-slot
    page. dma_gather pulls 64-f32 W rows (1024 events/call); DVE builds
    SegT[128ev,128slot] = is_equal(IC, sl) * sc  (sc = cj*ci, host-folded);
    PE: msgs.T @ SegT accumulated into a PSUM page [MU, 128].
    Pages -> SBUF stage (ACT, bf16) -> hT[d,k] = [MU, NSLOT] bf16 HBM (SYNC).
  Phase P: outT[d] [256, SH] = sum_rk fcblk_rk.T @ hT-slices + bias (f16 out).

Wire-format choices (the axon tunnel is ~50MB/s, so bytes dominate wall):
  gD[d,k]  int16 [16, NCALL*64]  -- un-replicated; device copies it into all
                                    eight 16-partition groups of the SBUF
                                    index buffer (dma_gather wants 8 replicas).
  slD[d]   uint8 [128, NWP]      -- slot-in-page, shared across the 3 k-passes.
  scD[d]   bf16  [128, NWP]      -- cj*ci edge scale, shared across k-passes.
  fcrT     bf16, hT staging bf16, outT float16.
Host assembles + transposes the two outputs (float32).
"""
import numpy as np
import ml_dtypes
import concourse.bass as bass
import concourse.bacc as bacc
import concourse.mybir as mybir

F32 = mybir.dt.float32
F16 = mybir.dt.float16
BF16 = mybir.dt.bfloat16
I16 = mybir.dt.int16
U8 = mybir.dt.uint8

NP_BF16 = ml_dtypes.bfloat16

R = 5
MU = 64
OUT = 256
NK = 3


class Cfg:
    def __init__(self, n_nodes, in_units, n_cores, wpp):
        self.N = n_nodes
        self.IN = in_units
        self.NC = n_cores
        self.SH = n_nodes // n_cores
        self.PPR = (self.SH + 127) // 128
        self.NPAGE = R * self.PPR
        self.NSLOT = self.NPAGE * 128
        self.WPP = wpp
        self.NW = self.NPAGE * wpp
        self.NWP = ((self.NW + 7) // 8) * 8
        self.NCALL = self.NWP // 8
        self.DCH = 512
        self.NDC = (self.SH + self.DCH - 1) // self.DCH
        self.SPS = 16
        self.NSTG = (self.NPAGE + self.SPS - 1) // self.SPS
        self.WROUND = (in_units * MU) // 2048
        assert (in_units * MU) % 2048 == 0


def host_prep_streams(cfg, feat_src, cj_src, ci_dst, src, dst, core):
    SH, PPR, WPP = cfg.SH, cfg.PPR, cfg.WPP
    lo, hi = core * SH, (core + 1) * SH
    evg = [[] for _ in range(NK)]
    evsl, evsc = [], []
    for r in range(R):
        m = (dst[r] >= lo) & (dst[r] < hi)
        s, d = src[r][m], dst[r][m]
        slot = r * PPR * 128 + (d - lo)
        sc = (cj_src[s, 0] * ci_dst[d, 0]).astype(np.float32)
        for k in range(NK):
            evg[k].append((r * cfg.IN + feat_src[s, k]).astype(np.int64))
        evsl.append(slot.astype(np.int64))
        evsc.append(sc)
    sl = np.concatenate(evsl)
    sc = np.concatenate(evsc)
    order = np.argsort(sl, kind="stable")
    sl, sc = sl[order], sc[order]
    NWP = cfg.NWP
    SL = np.zeros((NWP, 128), np.uint8)
    SC = np.zeros((NWP, 128), np.float32)
    page = sl // 128
    counts = np.bincount(page, minlength=cfg.NPAGE)
    assert counts.max() <= WPP * 128, (
        f"page overflow {counts.max()} > {WPP*128}; raise WPP")
    # window boundaries, shared by all three k streams
    bounds = []
    pos = 0
    for p in range(cfg.NPAGE):
        n = counts[p]
        sls, scs = sl[pos:pos+n], sc[pos:pos+n]
        for w in range((n + 127) // 128):
            a, b = w * 128, min((w + 1) * 128, n)
            wi = p * WPP + w
            SL[wi, :b-a] = (sls[a:b] - p * 128).astype(np.uint8)
            SC[wi, :b-a] = scs[a:b]
            bounds.append((wi, pos + a, pos + b))
        pos += n
    out = {
        "sl": SL.T.copy(),
        "sc": SC.T.astype(NP_BF16).copy(),
    }
    for k in range(NK):
        g = np.concatenate(evg[k])[order]
        G = np.zeros((NWP, 128), np.int16)
        for wi, a, b in bounds:
            G[wi, :b-a] = g[a:b]
        Gc = G.reshape(cfg.NCALL, 1024)
        wr = np.zeros((16, cfg.NCALL * 64), np.int16)
        for c in range(cfg.NCALL):
            wr[:, c*64:(c+1)*64] = Gc[c].reshape(64, 16).T
        out[f"g{k}"] = wr
    return out


def build_inputs(cfg, inputs):
    f32 = np.float32
    gi = lambda n: np.asarray(inputs[n], np.int64)
    gf = lambda n: np.asarray(inputs[n], f32)
    drug_feat, dis_feat = gi("drug_feat"), gi("dis_feat")
    src, dst = gi("src"), gi("dst")
    cj_drug, ci_drug = gf("cj_drug"), gf("ci_drug")
    cj_dis, ci_dis = gf("cj_dis"), gf("ci_dis")
    att, basis = gf("att"), gf("basis")
    fc_w, fc_b = gf("fc_w"), gf("fc_b")

    attT = att.T.copy()
    basisf = basis.reshape(4, cfg.IN * MU).copy()
    # fcrT[m, rk, o] = fc_w[r*NK*MU + k*MU + m, o]
    fcrT = fc_w.reshape(R * NK, MU, OUT).transpose(1, 0, 2).astype(NP_BF16).copy()
    fcb2 = fc_b.reshape(2, 128).T.copy()      # [128, 2], col h = half h
    IC = np.tile(np.arange(128, dtype=f32)[None, :], (128, 1)).copy()

    maps = []
    for core in range(cfg.NC):
        m = {"attT": attT, "basisf": basisf, "fcrT": fcrT, "fcb2": fcb2, "ic": IC}
        s0 = host_prep_streams(cfg, drug_feat, cj_drug, ci_dis, src, dst, core)
        s1 = host_prep_streams(cfg, dis_feat, cj_dis, ci_drug, dst, src, core)
        for d, s in ((0, s0), (1, s1)):
            m[f"d{d}sl"], m[f"d{d}sc"] = s["sl"], s["sc"]
            for k in range(NK):
                m[f"d{d}g{k}"] = s[f"g{k}"]
        maps.append(m)
    return maps


def assemble_output(cfg, results):
    dis_out = np.concatenate(
        [results[c]["outT"][0].T.astype(np.float32) for c in range(cfg.NC)], 0)
    drug_out = np.concatenate(
        [results[c]["outT"][1].T.astype(np.float32) for c in range(cfg.NC)], 0)
    return drug_out, dis_out


def build_kernel(cfg, debug=True):
    nc = bacc.Bacc(None, target_bir_lowering=False, debug=debug)
    IN, NCALL, NWP = cfg.IN, cfg.NCALL, cfg.NWP
    NPAGE, WPP, SH, PPR = cfg.NPAGE, cfg.WPP, cfg.SH, cfg.PPR
    NSLOT, DCH, NDC, SPS, NSTG = cfg.NSLOT, cfg.DCH, cfg.NDC, cfg.SPS, cfg.NSTG
    WROUND = cfg.WROUND
    PASSES = [(d, k) for d in range(2) for k in range(NK)]

    attT_d = nc.declare_dram_parameter("attT", [4, R], F32, isOutput=False)
    basisf_d = nc.declare_dram_parameter("basisf", [4, IN * MU], F32, isOutput=False)
    fcr_d = nc.declare_dram_parameter("fcrT", [MU, R * NK, OUT], BF16, isOutput=False)
    fcb_d = nc.declare_dram_parameter("fcb2", [128, 2], F32, isOutput=False)
    ic_d = nc.declare_dram_parameter("ic", [128, 128], F32, isOutput=False)
    gD, slD, scD = {}, {}, {}
    for d, k in PASSES:
        gD[d, k] = nc.declare_dram_parameter(f"d{d}g{k}", [16, NCALL * 64], I16, isOutput=False)
    for d in range(2):
        slD[d] = nc.declare_dram_parameter(f"d{d}sl", [128, NWP], U8, isOutput=False)
        scD[d] = nc.declare_dram_parameter(f"d{d}sc", [128, NWP], BF16, isOutput=False)
    outT_d = nc.declare_dram_parameter("outT", [2, OUT, SH], F16, isOutput=True)

    # wtab rows padded to 128 bf16 elems (256B) so dma_gather stays aligned;
    # cols 64:128 of each row are zero.
    wtab = nc.dram_tensor("wtab", [R, IN * 128], BF16)
    wtab_rows = wtab[:].rearrange("r (f m) -> (r f) m", m=128)
    hT = nc.dram_tensor("hT", [2, NK, MU, NSLOT], BF16)

    attT_sb = nc.alloc_sbuf_tensor("attT_sb", [4, R], F32)
    bchunk = nc.alloc_sbuf_tensor("bchunk", [4, 2048], F32)
    wstage = nc.alloc_sbuf_tensor("wstage", [R, 4096], BF16)
    ic_sb = nc.alloc_sbuf_tensor("ic_sb", [128, 128], F32)
    fcr_sb = nc.alloc_sbuf_tensor("fcr_sb", [MU, R * NK, OUT], BF16)
    fcb_sb = nc.alloc_sbuf_tensor("fcb_sb", [128, 2], F32)
    gsb = nc.alloc_sbuf_tensor("gsb", [128, NCALL * 64], I16)
    slr = nc.alloc_sbuf_tensor("slr", [128, NWP], U8)
    scr = nc.alloc_sbuf_tensor("scr", [128, NWP], BF16)
    slf = nc.alloc_sbuf_tensor("slf", [128, NWP], F32)
    scf = nc.alloc_sbuf_tensor("scf", [128, NWP], F32)
    NMB = 8
    msgs = [nc.alloc_sbuf_tensor(f"msgs{i}", [128, 8, 128], BF16) for i in range(NMB)]
    # segt is built page-at-a-time: WPP windows, plus the tail page which also
    # absorbs the NWP-NPAGE*WPP pad windows.
    WLAST = WPP + (cfg.NWP - cfg.NPAGE * cfg.WPP)
    NSB = 3
    segt = [nc.alloc_sbuf_tensor(f"segt{i}", [128, WLAST, 128], BF16)
            for i in range(NSB)]
    NSTB = 2
    stage = [nc.alloc_sbuf_tensor(f"stage{i}", [MU, SPS * 128], BF16) for i in range(NSTB)]
    prhs = [nc.alloc_sbuf_tensor(f"prhs{i}", [MU, R * NK, DCH], BF16) for i in range(2)]
    ostage = [nc.alloc_sbuf_tensor(f"ostage{i}", [128, DCH], F16) for i in range(4)]

    NPB = 4
    psA = nc.alloc_psum_tensor("psA", [128, 2048], F32)
    pages = [psA[0:MU, i * 512:i * 512 + 128] for i in range(NPB)]
    psB = nc.alloc_psum_tensor("psB", [128, 2048], F32)
    wps = psB[0:R, :]
    pps = [psB[:, j * 512:(j + 1) * 512] for j in range(4)]  # j = 2*(l%2)+h

    def page_of_window(w):
        return min(w // WPP, NPAGE - 1)

    wlast = {}
    for pi in range(len(PASSES)):
        for w in range(NWP):
            wlast[pi, page_of_window(w)] = pi * NWP + w

    sems = {}

    with nc.Block() as block:
        for name, n in [("gth", NMB), ("stg", NSTB), ("pin", 2), ("ost", 4)]:
            for i in range(n):
                sems[name, i] = nc.alloc_semaphore(f"s_{name}{i}")
        for name in ["wdma", "wout", "wmm", "wcp", "seg", "pe", "act", "pmm",
                     "oact", "gsb", "slraw"]:
            sems[name] = nc.alloc_semaphore(f"s_{name}")
        s_gth = [sems["gth", i] for i in range(NMB)]
        s_stg = [sems["stg", i] for i in range(NSTB)]
        s_pin = [sems["pin", i] for i in range(2)]
        s_ost = [sems["ost", i] for i in range(4)]
        s_wdma, s_wmm, s_wcp = sems["wdma"], sems["wmm"], sems["wcp"]
        s_wout = sems["wout"]
        s_seg, s_pe, s_act = sems["seg"], sems["pe"], sems["act"]
        s_pmm, s_oact = sems["pmm"], sems["oact"]
        s_gsb, s_slraw = sems["gsb"], sems["slraw"]

        # ============ GPSIMD: const + W-build DMAs, then gathers
        @block.gpsimd
        def _(g):
            g.dma_start(attT_sb[:], attT_d[:]).then_inc(s_wdma, 16)
            g.dma_start(ic_sb[:], ic_d[:]).then_inc(s_wdma, 16)
            g.dma_start(fcr_sb[:], fcr_d[:]).then_inc(s_wdma, 16)
            g.dma_start(fcb_sb[:], fcb_d[:]).then_inc(s_wdma, 16)
            for n in range(WROUND):
                g.wait_ge(s_wcp, n)  # round n-1 psum copied (wstage free after out-DMA below)
                g.dma_start(bchunk[:], basisf_d[:, n*2048:(n+1)*2048]).then_inc(s_wdma, 16)
                g.wait_ge(s_wcp, n + 1)
                g.dma_start(wtab[:, n*4096:(n+1)*4096], wstage[:]).then_inc(s_wout, 16)
            g.wait_ge(s_wout, WROUND * 16)  # all wtab writes landed
            ncall = 0
            for pi, (d, k) in enumerate(PASSES):
                g.wait_ge(s_gsb, 128 * (pi + 1))  # this pass's gsb loaded
                for c in range(NCALL):
                    b = ncall % NMB
                    if ncall >= NMB:
                        g.wait_ge(s_pe, 8 * (ncall - NMB + 1))
                    g.dma_gather(
                        msgs[b][:], wtab_rows,
                        gsb[:, c * 64:(c + 1) * 64],
                        1024, 1024, 128,
                    ).then_inc(s_gth[b], 16)
                    ncall += 1

        # ============ TENSOR: W MMs, window MMs, projection MMs
        @block.tensor
        def _(t):
            for n in range(WROUND):
                t.wait_ge(s_wdma, 64 + 16 * (n + 1))
                if n >= 1:
                    t.wait_ge(s_wcp, n)
                for i in range(4):
                    ins = t.matmul(wps[:, i*512:(i+1)*512], attT_sb[:],
                                   bchunk[:, i*512:(i+1)*512],
                                   start=True, stop=True)
                ins.then_inc(s_wmm, 1)
            wglob = 0
            for pi, (d, k) in enumerate(PASSES):
                for w in range(NWP):
                    p = page_of_window(w)
                    pglob = pi * NPAGE + p
                    first = (w % WPP == 0) and (p == w // WPP)
                    if first and pglob >= NPB:
                        t.wait_ge(s_act, pglob - NPB + 1)
                    b = (wglob // 8) % NMB
                    t.wait_ge(s_gth[b], 16 * (wglob // 8 // NMB + 1))
                    if first or w - p * WPP == 0:
                        t.wait_ge(s_seg, pglob + 1)
                    t.matmul(pages[pglob % NPB],
                             msgs[b][:, w % 8, 0:MU],
                             segt[pglob % NSB][:, w - p * WPP, :],
                             start=first, stop=(wglob == wlast[pi, p]),
                             ).then_inc(s_pe, 1)
                    wglob += 1
            nl = 0
            for d in range(2):
                for c in range(NDC):
                    ncols = min(DCH, SH - c * DCH)
                    t.wait_ge(s_pin[nl % 2], 240 * (nl // 2 + 1))
                    if nl >= 2:
                        t.wait_ge(s_oact, 2 * (nl - 1))
                    for h in range(2):
                        for rk in range(R * NK):
                            ins = t.matmul(pps[2*(nl % 2)+h][:, :ncols],
                                           fcr_sb[:, rk, h*128:(h+1)*128],
                                           prhs[nl % 2][:, rk, :ncols],
                                           start=(rk == 0), stop=(rk == R*NK-1))
                        ins.then_inc(s_pmm, 1)
                    nl += 1

        # ============ VECTOR: W psum->sbuf copies, sl/sc converts, SegT builds
        @block.vector
        def _(v):
            v.memset(wstage[:], 0)  # pad lanes 64:128 of each W row stay zero
            for n in range(WROUND):
                v.wait_ge(s_wmm, n + 1)
                if n >= 1:
                    v.wait_ge(s_wout, 16 * n)
                v.tensor_copy(
                    wstage[:].rearrange("r (f m) -> r f m", m=128)[:, :, 0:MU],
                    wps[:].rearrange("r (f m) -> r f m", m=MU),
                ).then_inc(s_wcp, 1)
            for pi, (d, k) in enumerate(PASSES):
                if k == 0:
                    # direction start: widen sl u8 / sc bf16 to f32 once
                    v.wait_ge(s_slraw, 32 * (d + 1))
                    v.tensor_copy(slf[:], slr[:])
                    v.tensor_copy(scf[:], scr[:])
                for p in range(NPAGE):
                    nwin = WPP if p < NPAGE - 1 else WLAST
                    pglob = pi * NPAGE + p
                    if pglob >= NSB:
                        pprev = pglob - NSB
                        v.wait_ge(s_pe, wlast[pprev // NPAGE, pprev % NPAGE] + 1)
                    sb = segt[pglob % NSB][:, 0:nwin, :]
                    ic_b = ic_sb[:].unsqueeze(1).to_broadcast([128, nwin, 128])
                    sl_b = slf[:, p*WPP:p*WPP+nwin].unsqueeze(2).to_broadcast(
                        [128, nwin, 128])
                    sc_b = scf[:, p*WPP:p*WPP+nwin].unsqueeze(2).to_broadcast(
                        [128, nwin, 128])
                    v.scalar_tensor_tensor(
                        sb, ic_b, 0.0, sl_b,
                        mybir.AluOpType.bypass, mybir.AluOpType.is_equal)
                    v.scalar_tensor_tensor(
                        sb, sb, 0.0, sc_b,
                        mybir.AluOpType.bypass, mybir.AluOpType.mult,
                    ).then_inc(s_seg, 1)
            nl = 0
            for d in range(2):
                for c in range(NDC):
                    ncols = min(DCH, SH - c * DCH)
                    for h in range(2):
                        ob = 2 * (nl % 2) + h
                        v.wait_ge(s_pmm, 2 * nl + h + 1)
                        if nl >= 2:
                            v.wait_ge(s_ost[ob], 16 * (nl // 2))
                        v.tensor_scalar(
                            ostage[ob][:, :ncols], pps[ob][:, :ncols],
                            fcb_sb[:, h:h+1], None,
                            mybir.AluOpType.add,
                        ).then_inc(s_oact, 1)
                    nl += 1

        # ============ SCALAR: page->stage copies (bf16); stage->hT DMAs
        @block.scalar
        def _(a):
            pglob = 0
            for pi, (d, k) in enumerate(PASSES):
                for p in range(NPAGE):
                    st = p // SPS
                    stglob = pi * NSTG + st
                    a.wait_ge(s_pe, wlast[pi, p] + 1)
                    if stglob >= NSTB and p % SPS == 0:
                        a.wait_ge(s_stg[stglob % NSTB], 16 * (stglob // NSTB))
                    a.copy(stage[stglob % NSTB][:, (p % SPS)*128:(p % SPS+1)*128],
                           pages[pglob % NPB]).then_inc(s_act, 1)
                    pglob += 1
                    if p % SPS == SPS - 1 or p == NPAGE - 1:
                        p0 = st * SPS
                        npg = p - p0 + 1
                        a.wait_ge(s_act, pglob)
                        a.dma_start(hT[d, k][:, p0*128:(p0+npg)*128],
                                    stage[stglob % NSTB][:, :npg*128]
                                    ).then_inc(s_stg[stglob % NSTB], 16)

        # ============ SYNC: g replication + sl/sc loads, proj loads, out DMAs
        @block.sync
        def _(s):
            for pi, (d, k) in enumerate(PASSES):
                if pi >= 1:
                    s.wait_ge(s_pe, pi * NWP)   # gsb (and for k==0, slr/scr) free
                if k == 0:
                    s.dma_start(slr[:], slD[d][:]).then_inc(s_slraw, 16)
                    s.dma_start(scr[:], scD[d][:]).then_inc(s_slraw, 16)
                for rep in range(8):
                    s.dma_start(gsb[rep*16:(rep+1)*16, :], gD[d, k][:]
                                ).then_inc(s_gsb, 16)
            # wait all stage->hT DMAs before projection loads
            NSTGALL = len(PASSES) * NSTG
            for b in range(NSTB):
                occ = (NSTGALL - b + NSTB - 1) // NSTB
                s.wait_ge(s_stg[b], 16 * occ)
            nl = 0
            for d in range(2):
                for c in range(NDC):
                    ncols = min(DCH, SH - c * DCH)
                    if nl >= 2:
                        s.wait_ge(s_pmm, 2 * (nl - 1))
                    for rk in range(R * NK):
                        r, k = rk // NK, rk % NK
                        s.dma_start(
                            prhs[nl % 2][:, rk, :ncols],
                            hT[d, k][:, r*PPR*128 + c*DCH: r*PPR*128 + c*DCH + ncols]
                        ).then_inc(s_pin[nl % 2], 16)
                    for h in range(2):
                        ob = 2 * (nl % 2) + h
                        s.wait_ge(s_oact, 2 * nl + h + 1)
                        s.dma_start(outT_d[d, h*128:(h+1)*128, c*DCH:c*DCH+ncols],
                                    ostage[ob][:, :ncols]).then_inc(s_ost[ob], 16)
                    nl += 1
            NLD = 2 * NDC
            for b in range(2):
                occ = (NLD - b + 1) // 2
                for h in range(2):
                    s.wait_ge(s_ost[2 * b + h], 16 * occ)

    nc.compile()
    return nc


# ======================================================================
# Self-contained kernel entry point.
# ======================================================================
from concourse.bass_utils import run_bass_kernel_spmd as _run_spmd

_CACHE = {}


def kernel(**inputs):
    """GCMC layer on 8 trn2 NeuronCores. Returns (drug_out, dis_out) f32."""
    cfg = Cfg(50000, 1024, 8, wpp=12)
    maps = build_inputs(cfg, inputs)
    if "nc" not in _CACHE:
        _CACHE["nc"] = build_kernel(cfg)
    res = _run_spmd(_CACHE["nc"], maps, list(range(cfg.NC)))
    return assemble_output(cfg, res.results)


# revision 38
# speedup vs baseline: 5.4776x; 1.0438x over previous
"""GCMC message-passing kernel for trn2: builder + host preprocessing.

Per core = one dst-shard, both directions (0: drug->dis, 1: dis->drug).
  Phase W: device computes W[r] = att @ basis -> wtab[R, IN*MU] f32 in HBM.
  Phase E (x6 passes = 2 dirs x 3 k-feats): per-edge event streams sorted by
    slot (r-major, dst-local), 128-event windows, WPP windows per 128-slot
    page. dma_gather pulls 64-f32 W rows (1024 events/call); DVE builds
    SegT[128ev,128slot] = is_equal(IC, sl) * sc  (sc = cj*ci, host-folded);
    PE: msgs.T @ SegT accumulated into a PSUM page [MU, 128].
    Pages -> SBUF stage (ACT, bf16) -> hT[d,k] = [MU, NSLOT] bf16 HBM (SYNC).
  Phase P: outT[d] [256, SH] = sum_rk fcblk_rk.T @ hT-slices + bias (f16 out).

Wire-format choices (the axon tunnel is ~50MB/s, so bytes dominate wall):
  gD[d,k]  int16 [16, NCALL*64]  -- un-replicated; device copies it into all
                                    eight 16-partition groups of the SBUF
                                    index buffer (dma_gather wants 8 replicas).
  slD[d]   uint8 [128, NWP]      -- slot-in-page, shared across the 3 k-passes.
  scD[d]   bf16  [128, NWP]      -- cj*ci edge scale, shared across k-passes.
  fcrT     bf16, hT staging bf16, outT float16.
Host assembles + transposes the two outputs (float32).
"""
import numpy as np
import ml_dtypes
import concourse.bass as bass
import concourse.bacc as bacc
import concourse.mybir as mybir

F32 = mybir.dt.float32
F16 = mybir.dt.float16
BF16 = mybir.dt.bfloat16
I16 = mybir.dt.int16
U8 = mybir.dt.uint8

NP_BF16 = ml_dtypes.bfloat16

R = 5
MU = 64
OUT = 256
NK = 3


class Cfg:
    def __init__(self, n_nodes, in_units, n_cores, wpp):
        self.N = n_nodes
        self.IN = in_units
        self.NC = n_cores
        self.SH = n_nodes // n_cores
        self.PPR = (self.SH + 127) // 128
        self.NPAGE = R * self.PPR
        self.NSLOT = self.NPAGE * 128
        self.WPP = wpp
        self.NW = self.NPAGE * wpp
        self.NWP = ((self.NW + 7) // 8) * 8
        self.NCALL = self.NWP // 8
        self.DCH = 512
        self.NDC = (self.SH + self.DCH - 1) // self.DCH
        self.SPS = 16
        self.NSTG = (self.NPAGE + self.SPS - 1) // self.SPS
        self.WROUND = (in_units * MU) // 2048
        assert (in_units * MU) % 2048 == 0


def host_prep_streams(cfg, feat_src, cj_src, ci_dst, src, dst, core):
    SH, PPR, WPP = cfg.SH, cfg.PPR, cfg.WPP
    lo, hi = core * SH, (core + 1) * SH
    evg = [[] for _ in range(NK)]
    evsl, evsc = [], []
    for r in range(R):
        m = (dst[r] >= lo) & (dst[r] < hi)
        s, d = src[r][m], dst[r][m]
        slot = r * PPR * 128 + (d - lo)
        sc = (cj_src[s, 0] * ci_dst[d, 0]).astype(np.float32)
        for k in range(NK):
            evg[k].append((r * cfg.IN + feat_src[s, k]).astype(np.int64))
        evsl.append(slot.astype(np.int64))
        evsc.append(sc)
    sl = np.concatenate(evsl)
    sc = np.concatenate(evsc)
    order = np.argsort(sl, kind="stable")
    sl, sc = sl[order], sc[order]
    NWP = cfg.NWP
    SL = np.zeros((NWP, 128), np.uint8)
    SC = np.zeros((NWP, 128), np.float32)
    page = sl // 128
    counts = np.bincount(page, minlength=cfg.NPAGE)
    assert counts.max() <= WPP * 128, (
        f"page overflow {counts.max()} > {WPP*128}; raise WPP")
    # window boundaries, shared by all three k streams
    bounds = []
    pos = 0
    for p in range(cfg.NPAGE):
        n = counts[p]
        sls, scs = sl[pos:pos+n], sc[pos:pos+n]
        for w in range((n + 127) // 128):
            a, b = w * 128, min((w + 1) * 128, n)
            wi = p * WPP + w
            SL[wi, :b-a] = (sls[a:b] - p * 128).astype(np.uint8)
            SC[wi, :b-a] = scs[a:b]
            bounds.append((wi, pos + a, pos + b))
        pos += n
    out = {
        "sl": SL.T.copy(),
        "sc": SC.T.astype(NP_BF16).copy(),
    }
    for k in range(NK):
        g = np.concatenate(evg[k])[order]
        G = np.zeros((NWP, 128), np.int16)
        for wi, a, b in bounds:
            G[wi, :b-a] = g[a:b]
        Gc = G.reshape(cfg.NCALL, 1024)
        wr = np.zeros((16, cfg.NCALL * 64), np.int16)
        for c in range(cfg.NCALL):
            wr[:, c*64:(c+1)*64] = Gc[c].reshape(64, 16).T
        out[f"g{k}"] = wr
    return out


def build_inputs(cfg, inputs):
    f32 = np.float32
    gi = lambda n: np.asarray(inputs[n], np.int64)
    gf = lambda n: np.asarray(inputs[n], f32)
    drug_feat, dis_feat = gi("drug_feat"), gi("dis_feat")
    src, dst = gi("src"), gi("dst")
    cj_drug, ci_drug = gf("cj_drug"), gf("ci_drug")
    cj_dis, ci_dis = gf("cj_dis"), gf("ci_dis")
    att, basis = gf("att"), gf("basis")
    fc_w, fc_b = gf("fc_w"), gf("fc_b")

    attT = att.T.astype(NP_BF16).copy()
    basisf = basis.reshape(4, cfg.IN * MU).astype(NP_BF16).copy()
    # fcrT[m, rk, o] = fc_w[r*NK*MU + k*MU + m, o]
    fcrT = fc_w.reshape(R * NK, MU, OUT).transpose(1, 0, 2).astype(NP_BF16).copy()
    fcb2 = fc_b.reshape(2, 128).T.copy()      # [128, 2], col h = half h
    IC = np.tile(np.arange(128, dtype=f32)[None, :], (128, 1)).copy()

    maps = []
    for core in range(cfg.NC):
        m = {"attT": attT, "basisf": basisf, "fcrT": fcrT, "fcb2": fcb2, "ic": IC}
        s0 = host_prep_streams(cfg, drug_feat, cj_drug, ci_dis, src, dst, core)
        s1 = host_prep_streams(cfg, dis_feat, cj_dis, ci_drug, dst, src, core)
        for d, s in ((0, s0), (1, s1)):
            m[f"d{d}sl"], m[f"d{d}sc"] = s["sl"], s["sc"]
            for k in range(NK):
                m[f"d{d}g{k}"] = s[f"g{k}"]
        maps.append(m)
    return maps


def assemble_output(cfg, results):
    dis_out = np.concatenate(
        [results[c]["outT"][0].T.astype(np.float32) for c in range(cfg.NC)], 0)
    drug_out = np.concatenate(
        [results[c]["outT"][1].T.astype(np.float32) for c in range(cfg.NC)], 0)
    return drug_out, dis_out


def build_kernel(cfg, debug=True):
    nc = bacc.Bacc(None, target_bir_lowering=False, debug=debug)
    IN, NCALL, NWP = cfg.IN, cfg.NCALL, cfg.NWP
    NPAGE, WPP, SH, PPR = cfg.NPAGE, cfg.WPP, cfg.SH, cfg.PPR
    NSLOT, DCH, NDC, SPS, NSTG = cfg.NSLOT, cfg.DCH, cfg.NDC, cfg.SPS, cfg.NSTG
    WROUND = cfg.WROUND
    PASSES = [(d, k) for d in range(2) for k in range(NK)]

    attT_d = nc.declare_dram_parameter("attT", [4, R], BF16, isOutput=False)
    basisf_d = nc.declare_dram_parameter("basisf", [4, IN * MU], BF16, isOutput=False)
    fcr_d = nc.declare_dram_parameter("fcrT", [MU, R * NK, OUT], BF16, isOutput=False)
    fcb_d = nc.declare_dram_parameter("fcb2", [128, 2], F32, isOutput=False)
    ic_d = nc.declare_dram_parameter("ic", [128, 128], F32, isOutput=False)
    gD, slD, scD = {}, {}, {}
    for d, k in PASSES:
        gD[d, k] = nc.declare_dram_parameter(f"d{d}g{k}", [16, NCALL * 64], I16, isOutput=False)
    for d in range(2):
        slD[d] = nc.declare_dram_parameter(f"d{d}sl", [128, NWP], U8, isOutput=False)
        scD[d] = nc.declare_dram_parameter(f"d{d}sc", [128, NWP], BF16, isOutput=False)
    outT_d = nc.declare_dram_parameter("outT", [2, OUT, SH], F16, isOutput=True)

    # wtab rows padded to 128 bf16 elems (256B) so dma_gather stays aligned;
    # cols 64:128 of each row are zero.
    wtab = nc.dram_tensor("wtab", [R, IN * 128], BF16)
    wtab_rows = wtab[:].rearrange("r (f m) -> (r f) m", m=128)
    hT = nc.dram_tensor("hT", [2, NK, MU, NSLOT], BF16)

    attT_sb = nc.alloc_sbuf_tensor("attT_sb", [4, R], BF16)
    bchunk = nc.alloc_sbuf_tensor("bchunk", [4, 2048], BF16)
    wstage = nc.alloc_sbuf_tensor("wstage", [R, 4096], BF16)
    ic_sb = nc.alloc_sbuf_tensor("ic_sb", [128, 128], F32)
    fcr_sb = nc.alloc_sbuf_tensor("fcr_sb", [MU, R * NK, OUT], BF16)
    fcb_sb = nc.alloc_sbuf_tensor("fcb_sb", [128, 2], F32)
    gsb = nc.alloc_sbuf_tensor("gsb", [128, NCALL * 64], I16)
    slr = nc.alloc_sbuf_tensor("slr", [128, NWP], U8)
    scr = nc.alloc_sbuf_tensor("scr", [128, NWP], BF16)
    slf = nc.alloc_sbuf_tensor("slf", [128, NWP], F32)
    scf = nc.alloc_sbuf_tensor("scf", [128, NWP], F32)
    NMB = 8
    msgs = [nc.alloc_sbuf_tensor(f"msgs{i}", [128, 8, 128], BF16) for i in range(NMB)]
    # segt is built page-at-a-time: WPP windows, plus the tail page which also
    # absorbs the NWP-NPAGE*WPP pad windows.
    WLAST = WPP + (cfg.NWP - cfg.NPAGE * cfg.WPP)
    NSB = 3
    segt = [nc.alloc_sbuf_tensor(f"segt{i}", [128, WLAST, 128], BF16)
            for i in range(NSB)]
    NSTB = 2
    stage = [nc.alloc_sbuf_tensor(f"stage{i}", [MU, SPS * 128], BF16) for i in range(NSTB)]
    prhs = [nc.alloc_sbuf_tensor(f"prhs{i}", [MU, R * NK, DCH], BF16) for i in range(2)]
    ostage = [nc.alloc_sbuf_tensor(f"ostage{i}", [128, DCH], F16) for i in range(4)]

    NPB = 4
    psA = nc.alloc_psum_tensor("psA", [128, 2048], F32)
    pages = [psA[0:MU, i * 512:i * 512 + 128] for i in range(NPB)]
    psB = nc.alloc_psum_tensor("psB", [128, 2048], F32)
    wps = psB[0:R, :]
    pps = [psB[:, j * 512:(j + 1) * 512] for j in range(4)]  # j = 2*(l%2)+h

    def page_of_window(w):
        return min(w // WPP, NPAGE - 1)

    wlast = {}
    for pi in range(len(PASSES)):
        for w in range(NWP):
            wlast[pi, page_of_window(w)] = pi * NWP + w

    sems = {}

    with nc.Block() as block:
        for name, n in [("gth", NMB), ("stg", NSTB), ("pin", 2), ("ost", 4)]:
            for i in range(n):
                sems[name, i] = nc.alloc_semaphore(f"s_{name}{i}")
        for name in ["wdma", "wout", "wmm", "wcp", "seg", "pe", "act", "pmm",
                     "oact", "gsb", "slraw"]:
            sems[name] = nc.alloc_semaphore(f"s_{name}")
        s_gth = [sems["gth", i] for i in range(NMB)]
        s_stg = [sems["stg", i] for i in range(NSTB)]
        s_pin = [sems["pin", i] for i in range(2)]
        s_ost = [sems["ost", i] for i in range(4)]
        s_wdma, s_wmm, s_wcp = sems["wdma"], sems["wmm"], sems["wcp"]
        s_wout = sems["wout"]
        s_seg, s_pe, s_act = sems["seg"], sems["pe"], sems["act"]
        s_pmm, s_oact = sems["pmm"], sems["oact"]
        s_gsb, s_slraw = sems["gsb"], sems["slraw"]

        # ============ GPSIMD: const + W-build DMAs, then gathers
        @block.gpsimd
        def _(g):
            g.dma_start(attT_sb[:], attT_d[:]).then_inc(s_wdma, 16)
            g.dma_start(ic_sb[:], ic_d[:]).then_inc(s_wdma, 16)
            g.dma_start(fcr_sb[:], fcr_d[:]).then_inc(s_wdma, 16)
            g.dma_start(fcb_sb[:], fcb_d[:]).then_inc(s_wdma, 16)
            for n in range(WROUND):
                g.wait_ge(s_wcp, n)  # round n-1 psum copied (wstage free after out-DMA below)
                g.dma_start(bchunk[:], basisf_d[:, n*2048:(n+1)*2048]).then_inc(s_wdma, 16)
                g.wait_ge(s_wcp, n + 1)
                g.dma_start(wtab[:, n*4096:(n+1)*4096], wstage[:]).then_inc(s_wout, 16)
            g.wait_ge(s_wout, WROUND * 16)  # all wtab writes landed
            ncall = 0
            for pi, (d, k) in enumerate(PASSES):
                g.wait_ge(s_gsb, 128 * (pi + 1))  # this pass's gsb loaded
                for c in range(NCALL):
                    b = ncall % NMB
                    if ncall >= NMB:
                        g.wait_ge(s_pe, 8 * (ncall - NMB + 1))
                    g.dma_gather(
                        msgs[b][:], wtab_rows,
                        gsb[:, c * 64:(c + 1) * 64],
                        1024, 1024, 128,
                    ).then_inc(s_gth[b], 16)
                    ncall += 1

        # ============ TENSOR: W MMs, window MMs, projection MMs
        @block.tensor
        def _(t):
            for n in range(WROUND):
                t.wait_ge(s_wdma, 64 + 16 * (n + 1))
                if n >= 1:
                    t.wait_ge(s_wcp, n)
                for i in range(4):
                    ins = t.matmul(wps[:, i*512:(i+1)*512], attT_sb[:],
                                   bchunk[:, i*512:(i+1)*512],
                                   start=True, stop=True)
                ins.then_inc(s_wmm, 1)
            wglob = 0
            for pi, (d, k) in enumerate(PASSES):
                for w in range(NWP):
                    p = page_of_window(w)
                    pglob = pi * NPAGE + p
                    first = (w % WPP == 0) and (p == w // WPP)
                    if first and pglob >= NPB:
                        t.wait_ge(s_act, pglob - NPB + 1)
                    b = (wglob // 8) % NMB
                    t.wait_ge(s_gth[b], 16 * (wglob // 8 // NMB + 1))
                    if first or w - p * WPP == 0:
                        t.wait_ge(s_seg, pglob + 1)
                    t.matmul(pages[pglob % NPB],
                             msgs[b][:, w % 8, 0:MU],
                             segt[pglob % NSB][:, w - p * WPP, :],
                             start=first, stop=(wglob == wlast[pi, p]),
                             ).then_inc(s_pe, 1)
                    wglob += 1
            nl = 0
            for d in range(2):
                for c in range(NDC):
                    ncols = min(DCH, SH - c * DCH)
                    t.wait_ge(s_pin[nl % 2], 240 * (nl // 2 + 1))
                    if nl >= 2:
                        t.wait_ge(s_oact, 2 * (nl - 1))
                    for h in range(2):
                        for rk in range(R * NK):
                            ins = t.matmul(pps[2*(nl % 2)+h][:, :ncols],
                                           fcr_sb[:, rk, h*128:(h+1)*128],
                                           prhs[nl % 2][:, rk, :ncols],
                                           start=(rk == 0), stop=(rk == R*NK-1))
                        ins.then_inc(s_pmm, 1)
                    nl += 1

        # ============ VECTOR: W psum->sbuf copies, sl/sc converts, SegT builds
        @block.vector
        def _(v):
            v.memset(wstage[:], 0)  # pad lanes 64:128 of each W row stay zero
            for n in range(WROUND):
                v.wait_ge(s_wmm, n + 1)
                if n >= 1:
                    v.wait_ge(s_wout, 16 * n)
                v.tensor_copy(
                    wstage[:].rearrange("r (f m) -> r f m", m=128)[:, :, 0:MU],
                    wps[:].rearrange("r (f m) -> r f m", m=MU),
                ).then_inc(s_wcp, 1)
            for pi, (d, k) in enumerate(PASSES):
                if k == 0:
                    # direction start: widen sl u8 / sc bf16 to f32 once
                    v.wait_ge(s_slraw, 32 * (d + 1))
                    v.tensor_copy(slf[:], slr[:])
                    v.tensor_copy(scf[:], scr[:])
                for p in range(NPAGE):
                    nwin = WPP if p < NPAGE - 1 else WLAST
                    pglob = pi * NPAGE + p
                    if pglob >= NSB:
                        pprev = pglob - NSB
                        v.wait_ge(s_pe, wlast[pprev // NPAGE, pprev % NPAGE] + 1)
                    sb = segt[pglob % NSB][:, 0:nwin, :]
                    ic_b = ic_sb[:].unsqueeze(1).to_broadcast([128, nwin, 128])
                    sl_b = slf[:, p*WPP:p*WPP+nwin].unsqueeze(2).to_broadcast(
                        [128, nwin, 128])
                    sc_b = scf[:, p*WPP:p*WPP+nwin].unsqueeze(2).to_broadcast(
                        [128, nwin, 128])
                    v.scalar_tensor_tensor(
                        sb, ic_b, 0.0, sl_b,
                        mybir.AluOpType.bypass, mybir.AluOpType.is_equal)
                    v.scalar_tensor_tensor(
                        sb, sb, 0.0, sc_b,
                        mybir.AluOpType.bypass, mybir.AluOpType.mult,
                    ).then_inc(s_seg, 1)
            nl = 0
            for d in range(2):
                for c in range(NDC):
                    ncols = min(DCH, SH - c * DCH)
                    for h in range(2):
                        ob = 2 * (nl % 2) + h
                        v.wait_ge(s_pmm, 2 * nl + h + 1)
                        if nl >= 2:
                            v.wait_ge(s_ost[ob], 16 * (nl // 2))
                        v.tensor_scalar(
                            ostage[ob][:, :ncols], pps[ob][:, :ncols],
                            fcb_sb[:, h:h+1], None,
                            mybir.AluOpType.add,
                        ).then_inc(s_oact, 1)
                    nl += 1

        # ============ SCALAR: page->stage copies (bf16); stage->hT DMAs
        @block.scalar
        def _(a):
            pglob = 0
            for pi, (d, k) in enumerate(PASSES):
                for p in range(NPAGE):
                    st = p // SPS
                    stglob = pi * NSTG + st
                    a.wait_ge(s_pe, wlast[pi, p] + 1)
                    if stglob >= NSTB and p % SPS == 0:
                        a.wait_ge(s_stg[stglob % NSTB], 16 * (stglob // NSTB))
                    a.copy(stage[stglob % NSTB][:, (p % SPS)*128:(p % SPS+1)*128],
                           pages[pglob % NPB]).then_inc(s_act, 1)
                    pglob += 1
                    if p % SPS == SPS - 1 or p == NPAGE - 1:
                        p0 = st * SPS
                        npg = p - p0 + 1
                        a.wait_ge(s_act, pglob)
                        a.dma_start(hT[d, k][:, p0*128:(p0+npg)*128],
                                    stage[stglob % NSTB][:, :npg*128]
                                    ).then_inc(s_stg[stglob % NSTB], 16)

        # ============ SYNC: g replication + sl/sc loads, proj loads, out DMAs
        @block.sync
        def _(s):
            for pi, (d, k) in enumerate(PASSES):
                if pi >= 1:
                    s.wait_ge(s_pe, pi * NWP)   # gsb (and for k==0, slr/scr) free
                if k == 0:
                    s.dma_start(slr[:], slD[d][:]).then_inc(s_slraw, 16)
                    s.dma_start(scr[:], scD[d][:]).then_inc(s_slraw, 16)
                for rep in range(8):
                    s.dma_start(gsb[rep*16:(rep+1)*16, :], gD[d, k][:]
                                ).then_inc(s_gsb, 16)
            # wait all stage->hT DMAs before projection loads
            NSTGALL = len(PASSES) * NSTG
            for b in range(NSTB):
                occ = (NSTGALL - b + NSTB - 1) // NSTB
                s.wait_ge(s_stg[b], 16 * occ)
            nl = 0
            for d in range(2):
                for c in range(NDC):
                    ncols = min(DCH, SH - c * DCH)
                    if nl >= 2:
                        s.wait_ge(s_pmm, 2 * (nl - 1))
                    for rk in range(R * NK):
                        r, k = rk // NK, rk % NK
                        s.dma_start(
                            prhs[nl % 2][:, rk, :ncols],
                            hT[d, k][:, r*PPR*128 + c*DCH: r*PPR*128 + c*DCH + ncols]
                        ).then_inc(s_pin[nl % 2], 16)
                    for h in range(2):
                        ob = 2 * (nl % 2) + h
                        s.wait_ge(s_oact, 2 * nl + h + 1)
                        s.dma_start(outT_d[d, h*128:(h+1)*128, c*DCH:c*DCH+ncols],
                                    ostage[ob][:, :ncols]).then_inc(s_ost[ob], 16)
                    nl += 1
            NLD = 2 * NDC
            for b in range(2):
                occ = (NLD - b + 1) // 2
                for h in range(2):
                    s.wait_ge(s_ost[2 * b + h], 16 * occ)

    nc.compile()
    return nc


# ======================================================================
# Self-contained kernel entry point.
# ======================================================================
from concourse.bass_utils import run_bass_kernel_spmd as _run_spmd

_CACHE = {}


def kernel(**inputs):
    """GCMC layer on 8 trn2 NeuronCores. Returns (drug_out, dis_out) f32."""
    cfg = Cfg(50000, 1024, 8, wpp=12)
    maps = build_inputs(cfg, inputs)
    if "nc" not in _CACHE:
        _CACHE["nc"] = build_kernel(cfg)
    res = _run_spmd(_CACHE["nc"], maps, list(range(cfg.NC)))
    return assemble_output(cfg, res.results)
